# revision 48
# baseline (speedup 1.0000x reference)
"""Trainium2 Bass kernel for ConvChebTemp (Chebyshev graph conv, temporal weights).

Math: out[b,v,o] = sum_{k,t,f} T_k(L)x0[:,t,f,b] w[f,k,t,o] + bias[o]
with x0 = inputs permuted to [V, T*Fin*B] and T_k the Chebyshev recurrence.

Clenshaw reformulation (weights contracted first):
  z_k[v,b,o] = sum_{t,f} x0[v,t,f,b] w[f,k,t,o]
  b3 = z3; b2 = z2 + 2 L b3; b1 = z1 + 2 L b2 - b3; out = z0 + L b1 - b2 + bias

Sharding: 8 cores = 4 pairs. Pair p owns batches [4p, 4p+4); within the pair
the graph rows are split in half (core 2p: rows [0, V/2), core 2p+1 the rest).
The Clenshaw iterates b3/b2/b1 live in pair-SHARED HBM tensors
(addr_space="Shared": cores (2k, 2k+1) see one physical buffer), so each
core writes only its half and gathers from the full tensor. Cross-core
ordering is a tiny per-pair AllGather barrier before each phase's gathers.

Everything on the SpMM path is bf16: gather rows are 4 batches x 64 Fout x 2B
= 512B (full DMA descriptor efficiency) and all matmuls run at 1 cycle/row.
"""
import sys

sys.path.insert(0, "/opt/trn_rl_repo")

from contextlib import ExitStack  # noqa: E402

import ml_dtypes  # noqa: E402
import numpy as np  # noqa: E402

from concourse import bacc, bass, mybir, tile  # noqa: E402
from concourse.bass_utils import run_bass_kernel_spmd  # noqa: E402

P = 128
N_CORES = 8
FP32 = mybir.dt.float32
BF16 = mybir.dt.bfloat16
I32 = mybir.dt.int32
I16 = mybir.dt.int16

# Problem dims (hardcoded per spec)
B, V, T, FIN = 16, 12288, 4, 64
KV, KT, FOUT = 4, 4, 64
VH = V // 2                # rows per core
NT = VH // P               # out-tiles per core (48)
BG = 4                     # batches per pair
F = BG * FOUT              # spmm row width (256 bf16 = 512B)
C = T * FIN                # z contraction dim (256)
PAIR_GROUPS = [[0, 1], [2, 3], [4, 5], [6, 7]]
CHUNKS_PER_PIECE = 8       # 1024 gather indices per instruction
DMA_SCRATCH = 16384        # SWDGE ring: 1024 descriptors
WGRP = 8                   # out-tiles per batched shared-HBM write


def _preprocess_lap(lap_rows, lap_cols, lap_vals):
    """Split nnz by row-half into own-column (section A) and peer-column
    (section B) chunk streams, padded to a common per-tile chunk structure
    (identical across cores so one SPMD program serves all).

    Section A only references rows this core wrote itself, so its gathers
    need no cross-core barrier.

    Returns (per_parity list of (gidx_wrapped, growl, gval),
    (chunksA_per_tile, chunksB_per_tile)).
    """
    halves = []
    cnt = np.zeros((2, 2, NT), np.int64)  # [section, parity, tile]
    for h in (0, 1):
        lo, hi = h * VH, (h + 1) * VH
        m = (lap_rows >= lo) & (lap_rows < hi)
        lrows = lap_rows[m] - lo
        order = np.argsort(lrows, kind="stable")
        lrows = lrows[order]
        cols = lap_cols[m][order]
        vals = lap_vals[m][order]
        own = (cols >= lo) & (cols < hi)
        tiles = lrows // P
        np.add.at(cnt[0, h], tiles[own], 1)
        np.add.at(cnt[1, h], tiles[~own], 1)
        halves.append((lrows, cols, vals, own, tiles))
    # pass A gets only FULL chunks of own-column nnz (min across parities so
    # neither pads); leftovers ride in pass B's first chunk, which is gathered
    # after the barrier anyway. This keeps total chunks near the unsplit count.
    chunksA = [min(int(cnt[0, 0][t] // P), int(cnt[0, 1][t] // P))
               for t in range(NT)]
    chunksB = [max(1,
                   int(-(-(cnt[0, 0][t] - P * chunksA[t] + cnt[1, 0][t]) // P)),
                   int(-(-(cnt[0, 1][t] - P * chunksA[t] + cnt[1, 1][t]) // P)))
               for t in range(NT)]
    nchunk = sum(chunksA) + sum(chunksB)
    nnzp = nchunk * P
    out = []
    for h, (lrows, cols, vals, own, tiles) in enumerate(halves):
        # pad slots must gather an own-half row (peer half may be unwritten
        # while section A streams): local row 0 of my half
        pad_v = h * VH
        gidx = np.full(nnzp, pad_v, np.int32)
        growl = np.zeros(nnzp, np.float32)
        gval = np.zeros(nnzp, np.float32)
        pos = 0
        for t in range(NT):  # section A: first P*chunksA[t] own nnz
            m = own & (tiles == t)
            n = P * chunksA[t]
            idx = np.flatnonzero(m)[:n]
            assert len(idx) == n
            gidx[pos:pos + n] = cols[idx]
            growl[pos:pos + n] = (lrows[idx] - t * P).astype(np.float32)
            gval[pos:pos + n] = vals[idx]
            pos += n
        for t in range(NT):  # section B: leftover own + all peer nnz
            m = own & (tiles == t)
            skip = P * chunksA[t]
            idx = np.concatenate([np.flatnonzero(m)[skip:],
                                  np.flatnonzero((~own) & (tiles == t))])
            n = len(idx)
            gidx[pos:pos + n] = cols[idx]
            growl[pos:pos + n] = (lrows[idx] - t * P).astype(np.float32)
            gval[pos:pos + n] = vals[idx]
            pos += chunksB[t] * P
        assert pos == nnzp
        # remap to partition-major rows: v -> (v % 128) * 96 + v // 128
        gidx = ((gidx % P) * (V // P) + gidx // P).astype(np.int16)
        gidx_w = np.tile(gidx.reshape(-1, 16).T.copy(), (8, 1))  # [128, nnzp/16]
        growl_m = growl.reshape(nchunk, P).T.copy()
        gval_m = gval.reshape(nchunk, P).T.copy()
        out.append((np.ascontiguousarray(gidx_w),
                    np.ascontiguousarray(growl_m),
                    np.ascontiguousarray(gval_m)))
    return out, (chunksA, chunksB)


def build_program(chunks_per_tile, has_bias, n_cores=N_CORES):
    nt = NT
    nchunk = sum(chunks_per_tile[0]) + sum(chunks_per_tile[1])
    nnzp = nchunk * P
    nc = bacc.Bacc("TRN2", target_bir_lowering=False, debug=False,
                   num_devices=n_cores, dynamic_dma_scratch_size=DMA_SCRATCH)

    xt_d = nc.dram_tensor("xt", [BG, 2, P, VH], BF16, kind="ExternalInput")
    wz_d = nc.dram_tensor("wz", [P, 2, KV * FOUT], BF16, kind="ExternalInput")
    onesb_d = nc.dram_tensor("onesb", [1, P], BF16, kind="ExternalInput")
    biasw_d = nc.dram_tensor("biasw", [1, KV * FOUT], BF16, kind="ExternalInput")
    iota_d = nc.dram_tensor("iota128", [P, P], BF16, kind="ExternalInput")
    ident_d = nc.dram_tensor("ident128", [P, P], BF16, kind="ExternalInput")
    nident_d = nc.dram_tensor("nident128", [P, P], BF16, kind="ExternalInput")
    offt_d = nc.dram_tensor("offt", [1, 1], I32, kind="ExternalInput")
    gidx_d = nc.dram_tensor("gidx", [P, nnzp // 16], I16, kind="ExternalInput")
    growl_d = nc.dram_tensor("growl", [P, nchunk], FP32, kind="ExternalInput")
    gval1_d = nc.dram_tensor("gval1", [P, nchunk], FP32, kind="ExternalInput")
    gval2_d = nc.dram_tensor("gval2", [P, nchunk], FP32, kind="ExternalInput")
    out_d = nc.dram_tensor("out", [P, NT, F], BF16, kind="ExternalOutput")

    # pair-shared Clenshaw iterates (both cores of a pair see one buffer),
    # stored partition-major: row v lives at [v % 128, v // 128, :] so the
    # per-core half writes are 128 contiguous 4KB descriptors per group
    bsh = [nc.dram_tensor(f"bsh{k}", [P, V // P, F], BF16, kind="Internal",
                          addr_space="Shared") for k in range(3)]
    bin_d = [nc.dram_tensor(f"bin{k}", [1, 16], BF16, kind="Internal")
             for k in range(3)]
    bout_d = [nc.dram_tensor(f"bout{k}", [2, 16], BF16, kind="Internal")
              for k in range(3)]

    with tile.TileContext(nc) as tc, ExitStack() as ctx:
        const = ctx.enter_context(tc.tile_pool(name="const", bufs=1))
        zres = ctx.enter_context(tc.tile_pool(name="zres", bufs=1))
        xpool = ctx.enter_context(tc.tile_pool(name="x", bufs=2))
        gpool = ctx.enter_context(tc.tile_pool(name="gbuf", bufs=4))
        spool = ctx.enter_context(tc.tile_pool(name="sel", bufs=6))
        opool = ctx.enter_context(tc.tile_pool(name="ostg", bufs=2))
        bpool = ctx.enter_context(tc.tile_pool(name="bounce", bufs=1))
        psz = ctx.enter_context(tc.tile_pool(name="psz", bufs=4, space="PSUM"))
        pss = ctx.enter_context(tc.tile_pool(name="pss", bufs=4, space="PSUM"))

        # constants + metadata resident in SBUF
        iota_sb = const.tile([P, P], BF16, tag="iota")
        nc.sync.dma_start(iota_sb[:], iota_d[:, :])
        ident_sb = const.tile([P, P], BF16, tag="ident")
        nc.sync.dma_start(ident_sb[:], ident_d[:, :])
        nident_sb = const.tile([P, P], BF16, tag="nident")
        nc.sync.dma_start(nident_sb[:], nident_d[:, :])
        ones_sb = const.tile([1, P], BF16, tag="ones")
        nc.sync.dma_start(ones_sb[:], onesb_d[:, :])
        biasw_sb = const.tile([1, KV * FOUT], BF16, tag="biasw")
        nc.sync.dma_start(biasw_sb[:], biasw_d[:, :])
        wz_sb = const.tile([P, 2, KV * FOUT], BF16, tag="wz")
        nc.sync.dma_start(wz_sb[:], wz_d[:, :, :])
        gidx_sb = const.tile([P, nnzp // 16], I16, tag="gidx")
        nc.sync.dma_start(gidx_sb[:], gidx_d[:, :])
        growl_sb = const.tile([P, nchunk], FP32, tag="growl")
        nc.sync.dma_start(growl_sb[:], growl_d[:, :])
        gval1_sb = const.tile([P, nchunk], FP32, tag="gval1")
        nc.sync.dma_start(gval1_sb[:], gval1_d[:, :])
        gval2_sb = const.tile([P, nchunk], FP32, tag="gval2")
        nc.sync.dma_start(gval2_sb[:], gval2_d[:, :])

        # my tile offset into the shared tensors (0 or NT); loaded on both
        # engines that issue symbolic shared writes (SP for the Z-phase b3
        # writes, Activation for the spmm-phase writes)
        off_by_eng = {}
        for eng, nm in ((nc.scalar, "act"), (nc.sync, "sp")):
            off_reg = eng.alloc_register(f"slab_off_{nm}")
            eng.reg_load(off_reg, offt_d[0:1, 0:1])
            off_by_eng[nm] = eng.snap(off_reg, donate=True, min_val=0,
                                      max_val=NT)

        # all z_k resident in SBUF: [P, nt, KV, BG, FOUT] bf16 (96KB/partition)
        z_sb = zres.tile([P, nt, KV, BG, FOUT], BF16, tag="z")

        shared_writes = {0: [], 1: [], 2: []}

        def write_half(kidx, kslot, grp, ntiles=WGRP):
            """Batched write of ntiles tiles of z-slot kslot to shared bsh[kidx]."""
            g0 = grp * WGRP
            eng, off = ((nc.sync, "sp") if kidx == 0 else (nc.scalar, "act"))
            dst = bsh[kidx][:, bass.ds(off_by_eng[off] + g0, ntiles), :]
            src = z_sb[:, g0:g0 + ntiles, kslot, :, :] \
                .rearrange("p t b o -> p t (b o)")
            w = eng.dma_start(dst, src)
            shared_writes[kidx].append(w)

        # ---------- phase Z: z_k = x0 @ w_k (+ bias folded into z0) ----------
        VHH = VH // 2
        for b in range(BG):
          for half in range(2):
            v0 = half * VHH
            xb = xpool.tile([P, 2, VHH], BF16, tag="xb")
            nc.sync.dma_start(
                xb[:], xt_d[b, :, :, v0:v0 + VHH].rearrange("c p v -> p c v"))
            for vt0 in range(half * nt // 2, (half + 1) * nt // 2, 2):
                zps = psz.tile([P, 2, KV * FOUT], FP32, tag="zps")
                for sub in range(2):
                    vt = vt0 + sub
                    for cc in range(2):
                        nc.tensor.matmul(
                            zps[:, sub, :],
                            lhsT=xb[:, cc, vt * P - v0:(vt + 1) * P - v0],
                            rhs=wz_sb[:, cc, :],
                            start=(cc == 0),
                            stop=(cc == 1 and not has_bias))
                    if has_bias:
                        nc.tensor.matmul(zps[:, sub, :], lhsT=ones_sb[:, :],
                                         rhs=biasw_sb[:, :], start=False,
                                         stop=True)
                # PSUM->SBUF cast copies: DVE 1/3, Act 2/3 (Act is cheaper)
                if (vt0 // 2) % 3 == 0:
                    nc.vector.tensor_copy(
                        z_sb[:, vt0:vt0 + 2, :, b, :],
                        zps[:].rearrange("p s (k o) -> p s k o", o=FOUT))
                else:
                    nc.scalar.activation(
                        out=z_sb[:, vt0:vt0 + 2, :, b, :],
                        in_=zps[:].rearrange("p s (k o) -> p s k o", o=FOUT),
                        func=mybir.ActivationFunctionType.Copy)
                if b == BG - 1 and (vt0 + 2) % WGRP == 0:
                    write_half(0, 3, vt0 // WGRP)

        def pair_barrier(k):
            # the AllGather is a pure rendezvous: gate it on ALL my shared
            # writes; completion proves the peer's writes are done too (the
            # payload itself is never read)
            cc = nc.gpsimd.collective_compute(
                "AllGather", mybir.AluOpType.bypass, PAIR_GROUPS,
                ins=[bin_d[k][0:1, :]], outs=[bout_d[k][:, :]])
            for w in shared_writes[k]:
                bass._add_dep_helper(cc.ins, w.ins, sync=True,
                                     reason="barrier after all shared writes")
            return cc

        # ---------- spmm phases ----------
        # Each phase runs in two passes: pass A covers own-half columns
        # (rows this core wrote -> no cross-core barrier; overlaps with the
        # AllGather rendezvous), pass B covers peer-half columns and waits
        # on the barrier. z-slots accumulate partials between the passes.
        chunksA, chunksB = chunks_per_tile
        nA = sum(chunksA)

        def spmm_pass(src_d, vals_sb, base0, chunks_list, nend, dep_inst,
                      seeds, finish):
            state = {"gb": None, "base": base0, "len": 0}

            def ensure_piece(c):
                while state["gb"] is None or c >= state["base"] + state["len"]:
                    base = (base0 if state["gb"] is None
                            else state["base"] + state["len"])
                    plen = min(CHUNKS_PER_PIECE, nend - base)
                    gb = gpool.tile([P, plen, F], BF16, tag="gb")
                    s0 = base * P
                    nidx = plen * P
                    g = nc.gpsimd.dma_gather(
                        out_ap=gb[:],
                        in_ap=src_d[:, :, :].rearrange("p t f -> (p t) f"),
                        idxs_ap=gidx_sb[:, s0 // 16:(s0 + nidx) // 16],
                        num_idxs=nidx,
                        num_idxs_reg=nidx,
                        elem_size=F,
                    )
                    if dep_inst is not None:
                        bass._add_dep_helper(g.ins, dep_inst.ins, sync=True,
                                             reason="pair barrier before gather")
                    state.update(gb=gb, base=base, len=plen)
                return state["gb"], state["base"]

            ci = base0
            for tt in range(nt):
                nck = chunks_list[tt]
                if nck == 0:
                    continue
                ps = pss.tile([P, F], FP32, tag="ps")
                # seed the accumulator with the running z-slot value(s) so
                # the DVE never has to do the adds
                sds = seeds(tt)
                for si, (w, src) in enumerate(sds):
                    nc.tensor.matmul(ps[:], lhsT=w, rhs=src,
                                     start=(si == 0), stop=False)
                for k in range(nck):
                    col = ci + k
                    gb, base = ensure_piece(col)
                    sT = spool.tile([P, P], BF16, tag="sT")
                    nc.vector.tensor_scalar(
                        out=sT[:], in0=iota_sb[:],
                        scalar1=growl_sb[:, col:col + 1],
                        scalar2=vals_sb[:, col:col + 1],
                        op0=mybir.AluOpType.is_equal,
                        op1=mybir.AluOpType.mult,
                    )
                    nc.tensor.matmul(ps[:], lhsT=sT[:], rhs=gb[:, col - base, :],
                                     start=False, stop=(k == nck - 1))
                finish(tt, ps)
                ci += nck

        def spmm_phase(src_d, vals_sb, cc_inst, seedsA, finishA, seedsB,
                       finishB):
            spmm_pass(src_d, vals_sb, 0, chunksA, nA, None, seedsA, finishA)
            spmm_pass(src_d, vals_sb, nA, chunksB, nchunk, cc_inst, seedsB,
                      finishB)

        def zslot(vt, k):
            return z_sb[:, vt, k, :, :].rearrange("p b o -> p (b o)")

        def ps3(ps):
            return ps[:].rearrange("p (b o) -> p b o", o=FOUT)

        def drain(tt, k, ps):
            nc.scalar.activation(out=zslot(tt, k), in_=ps[:],
                                 func=mybir.ActivationFunctionType.Copy)

        # phase 1: b2 = z2 + 2 L b3   (result overwrites z2 slot)
        cc0 = pair_barrier(0)

        def seeds1(tt):
            return [(ident_sb[:], zslot(tt, 2))]

        def finish1A(tt, ps):
            drain(tt, 2, ps)

        def finish1B(tt, ps):
            drain(tt, 2, ps)
            if tt == nt - WGRP // 2 - 1:
                write_half(1, 2, tt // WGRP, WGRP // 2)
            elif tt == nt - 1:
                g0 = nt - WGRP // 2
                w = nc.scalar.dma_start(
                    bsh[1][:, bass.ds(off_by_eng["act"] + g0, WGRP // 2), :],
                    z_sb[:, g0:g0 + WGRP // 2, 2, :, :]
                    .rearrange("p t b o -> p t (b o)"))
                shared_writes[1].append(w)
            elif (tt + 1) % WGRP == 0:
                write_half(1, 2, tt // WGRP)

        spmm_phase(bsh[0], gval2_sb, cc0, seeds1, finish1A, seeds1, finish1B)

        # phase 2: b1 = z1 + 2 L b2 - b3   (result overwrites z1 slot)
        cc1 = pair_barrier(1)

        def seeds2A(tt):
            return [(ident_sb[:], zslot(tt, 1))]

        def seeds2B(tt):
            return [(ident_sb[:], zslot(tt, 1)),
                    (nident_sb[:], zslot(tt, 3))]

        def finish2A(tt, ps):
            drain(tt, 1, ps)

        def finish2B(tt, ps):
            drain(tt, 1, ps)
            if tt == nt - WGRP // 2 - 1:
                write_half(2, 1, tt // WGRP, WGRP // 2)
            elif tt == nt - 1:
                g0 = nt - WGRP // 2
                w = nc.scalar.dma_start(
                    bsh[2][:, bass.ds(off_by_eng["act"] + g0, WGRP // 2), :],
                    z_sb[:, g0:g0 + WGRP // 2, 1, :, :]
                    .rearrange("p t b o -> p t (b o)"))
                shared_writes[2].append(w)
            elif (tt + 1) % WGRP == 0:
                write_half(2, 1, tt // WGRP)

        spmm_phase(bsh[1], gval2_sb, cc1, seeds2A, finish2A, seeds2B, finish2B)

        # phase 3: out = (z0 + L_own b1) + L_peer b1 - b2   (bias already in z0)
        cc2 = pair_barrier(2)

        def seeds3A(tt):
            return [(ident_sb[:], zslot(tt, 0))]

        def seeds3B(tt):
            return [(ident_sb[:], zslot(tt, 0)),
                    (nident_sb[:], zslot(tt, 2))]

        def finish3A(tt, ps):
            drain(tt, 0, ps)

        ostate = {"ot": None}

        def finish3B(tt, ps):
            if tt % WGRP == 0:
                ot_new = opool.tile([P, WGRP, F], BF16, tag="ot")
                ostate["ot"] = ot_new
            ot = ostate["ot"]
            nc.scalar.activation(out=ot[:, tt % WGRP, :], in_=ps[:],
                                 func=mybir.ActivationFunctionType.Copy)
            if (tt + 1) % WGRP == 0:
                g0 = (tt // WGRP) * WGRP
                nc.sync.dma_start(out_d[:, g0:g0 + WGRP, :], ot[:])

        spmm_phase(bsh[2], gval1_sb, cc2, seeds3A, finish3A, seeds3B, finish3B)

    nc.compile()
    return nc


def make_host_inputs(inputs, weight, bias, lap_vals, lap_rows, lap_cols):
    per_parity, chunks = _preprocess_lap(
        np.asarray(lap_rows), np.asarray(lap_cols),
        np.asarray(lap_vals, np.float32))
    w = np.asarray(weight, np.float32)
    # wz[(t,f) split cc, (k,o)]
    wz = np.transpose(w, (2, 0, 1, 3)).reshape(C, KV * FOUT)
    wz = np.ascontiguousarray(
        wz.reshape(2, P, KV * FOUT).transpose(1, 0, 2)).astype(ml_dtypes.bfloat16)
    biasw = np.zeros((1, KV * FOUT), np.float32)
    biasw[0, :FOUT] = np.asarray(bias, np.float32)
    biasw = biasw.astype(ml_dtypes.bfloat16)
    onesb = np.ones((1, P), ml_dtypes.bfloat16)
    ident128 = np.eye(P, dtype=np.float32).astype(ml_dtypes.bfloat16)
    iota128 = np.ascontiguousarray(
        np.broadcast_to(np.arange(P, dtype=np.float32)[None, :],
                        (P, P))).astype(ml_dtypes.bfloat16)
    x = np.asarray(inputs, np.float32)
    in_maps = []
    for r in range(N_CORES):
        pair, h = r // 2, r % 2
        gidx_w, growl_m, gval_m = per_parity[h]
        # xt[b, cc, cl, v] = x[4p+b, h*VH + v, t, f], c=(t,f)=cc*128+cl
        xs = x[BG * pair:BG * (pair + 1), h * VH:(h + 1) * VH]  # [4, VH, T, FIN]
        xt = xs.reshape(BG, VH, C).transpose(0, 2, 1).reshape(BG, 2, P, VH)
        m = {
            "xt": np.ascontiguousarray(xt).astype(ml_dtypes.bfloat16),
            "wz": wz,
            "biasw": biasw,
            "onesb": onesb,
            "iota128": iota128,
            "ident128": ident128,
            "nident128": -ident128,
            "offt": np.array([[h * NT]], np.int32),
            "gidx": gidx_w,
            "growl": growl_m,
            "gval1": gval_m,
            "gval2": np.ascontiguousarray(2.0 * gval_m),
        }
        in_maps.append(m)
    return in_maps, chunks


_CACHE = {}


def _get_program(chunks, has_bias):
    key = (tuple(chunks[0]), tuple(chunks[1]), has_bias)
    if key not in _CACHE:
        _CACHE[key] = build_program((list(chunks[0]), list(chunks[1])), has_bias)
    return _CACHE[key]


def kernel(inputs, weight, bias, lap_vals, lap_rows, lap_cols):
    in_maps, chunks = make_host_inputs(inputs, weight, bias, lap_vals,
                                       lap_rows, lap_cols)
    nc = _get_program(chunks, bool(np.any(np.asarray(bias))))
    res = run_bass_kernel_spmd(nc, in_maps, list(range(N_CORES)))
    out = np.empty((B, V, FOUT), np.float32)
    for r in range(N_CORES):
        pair, h = r // 2, r % 2
        o = np.asarray(res.results[r]["out"], np.float32).reshape(P, NT, BG, FOUT)
        o = o.transpose(2, 1, 0, 3).reshape(BG, VH, FOUT)
        out[BG * pair:BG * (pair + 1), h * VH:(h + 1) * VH, :] = o
    return np.ascontiguousarray(out)


def time_kernel(inputs_dict, iters=3):
    """Wall-clock repeated executions of the cached program (ns per run)."""
    import time

    in_maps, chunks = make_host_inputs(**inputs_dict)
    nc = _get_program(chunks, bool(np.any(np.asarray(inputs_dict["bias"]))))
    times = []
    for _ in range(iters):
        t0 = time.perf_counter()
        run_bass_kernel_spmd(nc, in_maps, list(range(N_CORES)))
        times.append(time.perf_counter() - t0)
    return min(times) * 1e9


# revision 49
# speedup vs baseline: 1.0025x; 1.0025x over previous
"""Trainium2 Bass kernel for ConvChebTemp (Chebyshev graph conv, temporal weights).

Math: out[b,v,o] = sum_{k,t,f} T_k(L)x0[:,t,f,b] w[f,k,t,o] + bias[o]
with x0 = inputs permuted to [V, T*Fin*B] and T_k the Chebyshev recurrence.

Clenshaw reformulation (weights contracted first):
  z_k[v,b,o] = sum_{t,f} x0[v,t,f,b] w[f,k,t,o]
  b3 = z3; b2 = z2 + 2 L b3; b1 = z1 + 2 L b2 - b3; out = z0 + L b1 - b2 + bias

Sharding: 8 cores = 4 pairs. Pair p owns batches [4p, 4p+4); within the pair
the graph rows are split in half (core 2p: rows [0, V/2), core 2p+1 the rest).
The Clenshaw iterates b3/b2/b1 live in pair-SHARED HBM tensors
(addr_space="Shared": cores (2k, 2k+1) see one physical buffer), so each
core writes only its half and gathers from the full tensor. Cross-core
ordering is a tiny per-pair AllGather barrier before each phase's gathers.

Everything on the SpMM path is bf16: gather rows are 4 batches x 64 Fout x 2B
= 512B (full DMA descriptor efficiency) and all matmuls run at 1 cycle/row.
"""
import sys

sys.path.insert(0, "/opt/trn_rl_repo")

from contextlib import ExitStack  # noqa: E402

import ml_dtypes  # noqa: E402
import numpy as np  # noqa: E402

from concourse import bacc, bass, mybir, tile  # noqa: E402
from concourse.bass_utils import run_bass_kernel_spmd  # noqa: E402

P = 128
N_CORES = 8
FP32 = mybir.dt.float32
BF16 = mybir.dt.bfloat16
I32 = mybir.dt.int32
I16 = mybir.dt.int16

# Problem dims (hardcoded per spec)
B, V, T, FIN = 16, 12288, 4, 64
KV, KT, FOUT = 4, 4, 64
VH = V // 2                # rows per core
NT = VH // P               # out-tiles per core (48)
BG = 4                     # batches per pair
F = BG * FOUT              # spmm row width (256 bf16 = 512B)
C = T * FIN                # z contraction dim (256)
PAIR_GROUPS = [[0, 1], [2, 3], [4, 5], [6, 7]]
CHUNKS_PER_PIECE = 8       # 1024 gather indices per instruction
DMA_SCRATCH = 16384        # SWDGE ring: 1024 descriptors
WGRP = 8                   # out-tiles per batched shared-HBM write


def _preprocess_lap(lap_rows, lap_cols, lap_vals):
    """Split nnz by row-half into own-column (section A) and peer-column
    (section B) chunk streams, padded to a common per-tile chunk structure
    (identical across cores so one SPMD program serves all).

    Section A only references rows this core wrote itself, so its gathers
    need no cross-core barrier.

    Returns (per_parity list of (gidx_wrapped, growl, gval),
    (chunksA_per_tile, chunksB_per_tile)).
    """
    halves = []
    cnt = np.zeros((2, 2, NT), np.int64)  # [section, parity, tile]
    for h in (0, 1):
        lo, hi = h * VH, (h + 1) * VH
        m = (lap_rows >= lo) & (lap_rows < hi)
        lrows = lap_rows[m] - lo
        order = np.argsort(lrows, kind="stable")
        lrows = lrows[order]
        cols = lap_cols[m][order]
        vals = lap_vals[m][order]
        own = (cols >= lo) & (cols < hi)
        tiles = lrows // P
        np.add.at(cnt[0, h], tiles[own], 1)
        np.add.at(cnt[1, h], tiles[~own], 1)
        halves.append((lrows, cols, vals, own, tiles))
    # pass A gets only FULL chunks of own-column nnz (min across parities so
    # neither pads); leftovers ride in pass B's first chunk, which is gathered
    # after the barrier anyway. This keeps total chunks near the unsplit count.
    chunksA = [min(int(cnt[0, 0][t] // P), int(cnt[0, 1][t] // P))
               for t in range(NT)]
    chunksB = [max(1,
                   int(-(-(cnt[0, 0][t] - P * chunksA[t] + cnt[1, 0][t]) // P)),
                   int(-(-(cnt[0, 1][t] - P * chunksA[t] + cnt[1, 1][t]) // P)))
               for t in range(NT)]
    nchunk = sum(chunksA) + sum(chunksB)
    nnzp = nchunk * P
    out = []
    for h, (lrows, cols, vals, own, tiles) in enumerate(halves):
        # pad slots must gather an own-half row (peer half may be unwritten
        # while section A streams): local row 0 of my half
        pad_v = h * VH
        gidx = np.full(nnzp, pad_v, np.int32)
        growl = np.zeros(nnzp, np.float32)
        gval = np.zeros(nnzp, np.float32)
        pos = 0
        for t in range(NT):  # section A: first P*chunksA[t] own nnz
            m = own & (tiles == t)
            n = P * chunksA[t]
            idx = np.flatnonzero(m)[:n]
            assert len(idx) == n
            gidx[pos:pos + n] = cols[idx]
            growl[pos:pos + n] = (lrows[idx] - t * P).astype(np.float32)
            gval[pos:pos + n] = vals[idx]
            pos += n
        for t in range(NT):  # section B: leftover own + all peer nnz
            m = own & (tiles == t)
            skip = P * chunksA[t]
            idx = np.concatenate([np.flatnonzero(m)[skip:],
                                  np.flatnonzero((~own) & (tiles == t))])
            n = len(idx)
            gidx[pos:pos + n] = cols[idx]
            growl[pos:pos + n] = (lrows[idx] - t * P).astype(np.float32)
            gval[pos:pos + n] = vals[idx]
            pos += chunksB[t] * P
        assert pos == nnzp
        # remap to partition-major rows: v -> (v % 128) * 96 + v // 128
        gidx = ((gidx % P) * (V // P) + gidx // P).astype(np.int16)
        gidx_w = np.tile(gidx.reshape(-1, 16).T.copy(), (8, 1))  # [128, nnzp/16]
        growl_m = growl.reshape(nchunk, P).T.copy()
        gval_m = gval.reshape(nchunk, P).T.copy()
        out.append((np.ascontiguousarray(gidx_w),
                    np.ascontiguousarray(growl_m),
                    np.ascontiguousarray(gval_m)))
    return out, (chunksA, chunksB)


def build_program(chunks_per_tile, has_bias, n_cores=N_CORES):
    nt = NT
    nchunk = sum(chunks_per_tile[0]) + sum(chunks_per_tile[1])
    nnzp = nchunk * P
    nc = bacc.Bacc("TRN2", target_bir_lowering=False, debug=False,
                   num_devices=n_cores, dynamic_dma_scratch_size=DMA_SCRATCH)

    xt_d = nc.dram_tensor("xt", [BG, 2, P, VH], BF16, kind="ExternalInput")
    wz_d = nc.dram_tensor("wz", [P, 2, KV * FOUT], BF16, kind="ExternalInput")
    onesb_d = nc.dram_tensor("onesb", [1, P], BF16, kind="ExternalInput")
    biasw_d = nc.dram_tensor("biasw", [1, KV * FOUT], BF16, kind="ExternalInput")
    iota_d = nc.dram_tensor("iota128", [P, P], BF16, kind="ExternalInput")
    ident_d = nc.dram_tensor("ident128", [P, P], BF16, kind="ExternalInput")
    nident_d = nc.dram_tensor("nident128", [P, P], BF16, kind="ExternalInput")
    offt_d = nc.dram_tensor("offt", [1, 1], I32, kind="ExternalInput")
    gidx_d = nc.dram_tensor("gidx", [P, nnzp // 16], I16, kind="ExternalInput")
    growl_d = nc.dram_tensor("growl", [P, nchunk], FP32, kind="ExternalInput")
    gval1_d = nc.dram_tensor("gval1", [P, nchunk], FP32, kind="ExternalInput")
    gval2_d = nc.dram_tensor("gval2", [P, nchunk], FP32, kind="ExternalInput")
    out_d = nc.dram_tensor("out", [P, NT, F], BF16, kind="ExternalOutput")

    # pair-shared Clenshaw iterates (both cores of a pair see one buffer),
    # stored partition-major: row v lives at [v % 128, v // 128, :] so the
    # per-core half writes are 128 contiguous 4KB descriptors per group
    bsh = [nc.dram_tensor(f"bsh{k}", [P, V // P, F], BF16, kind="Internal",
                          addr_space="Shared") for k in range(3)]
    bin_d = [nc.dram_tensor(f"bin{k}", [1, 16], BF16, kind="Internal")
             for k in range(3)]
    bout_d = [nc.dram_tensor(f"bout{k}", [2, 16], BF16, kind="Internal")
              for k in range(3)]

    with tile.TileContext(nc) as tc, ExitStack() as ctx:
        const = ctx.enter_context(tc.tile_pool(name="const", bufs=1))
        zres = ctx.enter_context(tc.tile_pool(name="zres", bufs=1))
        xpool = ctx.enter_context(tc.tile_pool(name="x", bufs=2))
        gpool = ctx.enter_context(tc.tile_pool(name="gbuf", bufs=4))
        spool = ctx.enter_context(tc.tile_pool(name="sel", bufs=4))
        opool = ctx.enter_context(tc.tile_pool(name="ostg", bufs=2))
        bpool = ctx.enter_context(tc.tile_pool(name="bounce", bufs=1))
        psz = ctx.enter_context(tc.tile_pool(name="psz", bufs=3, space="PSUM"))
        pss = ctx.enter_context(tc.tile_pool(name="pss", bufs=5, space="PSUM"))

        # constants + metadata resident in SBUF
        iota_sb = const.tile([P, P], BF16, tag="iota")
        nc.sync.dma_start(iota_sb[:], iota_d[:, :])
        ident_sb = const.tile([P, P], BF16, tag="ident")
        nc.sync.dma_start(ident_sb[:], ident_d[:, :])
        nident_sb = const.tile([P, P], BF16, tag="nident")
        nc.sync.dma_start(nident_sb[:], nident_d[:, :])
        ones_sb = const.tile([1, P], BF16, tag="ones")
        nc.sync.dma_start(ones_sb[:], onesb_d[:, :])
        biasw_sb = const.tile([1, KV * FOUT], BF16, tag="biasw")
        nc.sync.dma_start(biasw_sb[:], biasw_d[:, :])
        wz_sb = const.tile([P, 2, KV * FOUT], BF16, tag="wz")
        nc.sync.dma_start(wz_sb[:], wz_d[:, :, :])
        gidx_sb = const.tile([P, nnzp // 16], I16, tag="gidx")
        nc.sync.dma_start(gidx_sb[:], gidx_d[:, :])
        growl_sb = const.tile([P, nchunk], FP32, tag="growl")
        nc.sync.dma_start(growl_sb[:], growl_d[:, :])
        gval1_sb = const.tile([P, nchunk], FP32, tag="gval1")
        nc.sync.dma_start(gval1_sb[:], gval1_d[:, :])
        gval2_sb = const.tile([P, nchunk], FP32, tag="gval2")
        nc.sync.dma_start(gval2_sb[:], gval2_d[:, :])

        # my tile offset into the shared tensors (0 or NT); loaded on both
        # engines that issue symbolic shared writes (SP for the Z-phase b3
        # writes, Activation for the spmm-phase writes)
        off_by_eng = {}
        for eng, nm in ((nc.scalar, "act"), (nc.sync, "sp")):
            off_reg = eng.alloc_register(f"slab_off_{nm}")
            eng.reg_load(off_reg, offt_d[0:1, 0:1])
            off_by_eng[nm] = eng.snap(off_reg, donate=True, min_val=0,
                                      max_val=NT)

        # all z_k resident in SBUF: [P, nt, KV, BG, FOUT] bf16 (96KB/partition)
        z_sb = zres.tile([P, nt, KV, BG, FOUT], BF16, tag="z")

        shared_writes = {0: [], 1: [], 2: []}

        def write_half(kidx, kslot, grp, ntiles=WGRP):
            """Batched write of ntiles tiles of z-slot kslot to shared bsh[kidx]."""
            g0 = grp * WGRP
            eng, off = ((nc.sync, "sp") if kidx == 0 else (nc.scalar, "act"))
            dst = bsh[kidx][:, bass.ds(off_by_eng[off] + g0, ntiles), :]
            src = z_sb[:, g0:g0 + ntiles, kslot, :, :] \
                .rearrange("p t b o -> p t (b o)")
            w = eng.dma_start(dst, src)
            shared_writes[kidx].append(w)

        # ---------- phase Z: z_k = x0 @ w_k (+ bias folded into z0) ----------
        VHH = VH // 2
        for b in range(BG):
          for half in range(2):
            v0 = half * VHH
            xb = xpool.tile([P, 2, VHH], BF16, tag="xb")
            nc.sync.dma_start(
                xb[:], xt_d[b, :, :, v0:v0 + VHH].rearrange("c p v -> p c v"))
            for vt0 in range(half * nt // 2, (half + 1) * nt // 2, 2):
                zps = psz.tile([P, 2, KV * FOUT], FP32, tag="zps")
                for sub in range(2):
                    vt = vt0 + sub
                    for cc in range(2):
                        nc.tensor.matmul(
                            zps[:, sub, :],
                            lhsT=xb[:, cc, vt * P - v0:(vt + 1) * P - v0],
                            rhs=wz_sb[:, cc, :],
                            start=(cc == 0),
                            stop=(cc == 1 and not has_bias))
                    if has_bias:
                        nc.tensor.matmul(zps[:, sub, :], lhsT=ones_sb[:, :],
                                         rhs=biasw_sb[:, :], start=False,
                                         stop=True)
                # PSUM->SBUF cast copies: DVE 1/3, Act 2/3 (Act is cheaper)
                if (vt0 // 2) % 3 == 0:
                    nc.vector.tensor_copy(
                        z_sb[:, vt0:vt0 + 2, :, b, :],
                        zps[:].rearrange("p s (k o) -> p s k o", o=FOUT))
                else:
                    nc.scalar.activation(
                        out=z_sb[:, vt0:vt0 + 2, :, b, :],
                        in_=zps[:].rearrange("p s (k o) -> p s k o", o=FOUT),
                        func=mybir.ActivationFunctionType.Copy)
                if b == BG - 1 and (vt0 + 2) % WGRP == 0:
                    write_half(0, 3, vt0 // WGRP)

        def pair_barrier(k):
            # the AllGather is a pure rendezvous: gate it on ALL my shared
            # writes; completion proves the peer's writes are done too (the
            # payload itself is never read)
            cc = nc.gpsimd.collective_compute(
                "AllGather", mybir.AluOpType.bypass, PAIR_GROUPS,
                ins=[bin_d[k][0:1, :]], outs=[bout_d[k][:, :]])
            for w in shared_writes[k]:
                bass._add_dep_helper(cc.ins, w.ins, sync=True,
                                     reason="barrier after all shared writes")
            return cc

        # ---------- spmm phases ----------
        # Each phase runs in two passes: pass A covers own-half columns
        # (rows this core wrote -> no cross-core barrier; overlaps with the
        # AllGather rendezvous), pass B covers peer-half columns and waits
        # on the barrier. z-slots accumulate partials between the passes.
        chunksA, chunksB = chunks_per_tile
        nA = sum(chunksA)

        def spmm_pass(src_d, vals_sb, base0, chunks_list, nend, dep_inst,
                      seeds, finish):
            state = {"gb": None, "base": base0, "len": 0}

            def ensure_piece(c):
                while state["gb"] is None or c >= state["base"] + state["len"]:
                    base = (base0 if state["gb"] is None
                            else state["base"] + state["len"])
                    plen = min(CHUNKS_PER_PIECE, nend - base)
                    gb = gpool.tile([P, plen, F], BF16, tag="gb")
                    s0 = base * P
                    nidx = plen * P
                    g = nc.gpsimd.dma_gather(
                        out_ap=gb[:],
                        in_ap=src_d[:, :, :].rearrange("p t f -> (p t) f"),
                        idxs_ap=gidx_sb[:, s0 // 16:(s0 + nidx) // 16],
                        num_idxs=nidx,
                        num_idxs_reg=nidx,
                        elem_size=F,
                    )
                    if dep_inst is not None:
                        bass._add_dep_helper(g.ins, dep_inst.ins, sync=True,
                                             reason="pair barrier before gather")
                    state.update(gb=gb, base=base, len=plen)
                return state["gb"], state["base"]

            ci = base0
            for tt in range(nt):
                nck = chunks_list[tt]
                if nck == 0:
                    continue
                ps = pss.tile([P, F], FP32, tag="ps")
                # seed the accumulator with the running z-slot value(s) so
                # the DVE never has to do the adds
                sds = seeds(tt)
                for si, (w, src) in enumerate(sds):
                    nc.tensor.matmul(ps[:], lhsT=w, rhs=src,
                                     start=(si == 0), stop=False)
                for k in range(nck):
                    col = ci + k
                    gb, base = ensure_piece(col)
                    sT = spool.tile([P, P], BF16, tag="sT")
                    nc.vector.tensor_scalar(
                        out=sT[:], in0=iota_sb[:],
                        scalar1=growl_sb[:, col:col + 1],
                        scalar2=vals_sb[:, col:col + 1],
                        op0=mybir.AluOpType.is_equal,
                        op1=mybir.AluOpType.mult,
                    )
                    nc.tensor.matmul(ps[:], lhsT=sT[:], rhs=gb[:, col - base, :],
                                     start=False, stop=(k == nck - 1))
                finish(tt, ps)
                ci += nck

        def spmm_phase(src_d, vals_sb, cc_inst, seedsA, finishA, seedsB,
                       finishB):
            spmm_pass(src_d, vals_sb, 0, chunksA, nA, None, seedsA, finishA)
            spmm_pass(src_d, vals_sb, nA, chunksB, nchunk, cc_inst, seedsB,
                      finishB)

        def zslot(vt, k):
            return z_sb[:, vt, k, :, :].rearrange("p b o -> p (b o)")

        def ps3(ps):
            return ps[:].rearrange("p (b o) -> p b o", o=FOUT)

        def drain(tt, k, ps):
            nc.scalar.activation(out=zslot(tt, k), in_=ps[:],
                                 func=mybir.ActivationFunctionType.Copy)

        # phase 1: b2 = z2 + 2 L b3   (result overwrites z2 slot)
        cc0 = pair_barrier(0)

        def seeds1(tt):
            return [(ident_sb[:], zslot(tt, 2))]

        def finish1A(tt, ps):
            drain(tt, 2, ps)

        def finish1B(tt, ps):
            drain(tt, 2, ps)
            if tt == nt - WGRP // 2 - 1:
                write_half(1, 2, tt // WGRP, WGRP // 2)
            elif tt == nt - 1:
                g0 = nt - WGRP // 2
                w = nc.scalar.dma_start(
                    bsh[1][:, bass.ds(off_by_eng["act"] + g0, WGRP // 2), :],
                    z_sb[:, g0:g0 + WGRP // 2, 2, :, :]
                    .rearrange("p t b o -> p t (b o)"))
                shared_writes[1].append(w)
            elif (tt + 1) % WGRP == 0:
                write_half(1, 2, tt // WGRP)

        spmm_phase(bsh[0], gval2_sb, cc0, seeds1, finish1A, seeds1, finish1B)

        # phase 2: b1 = z1 + 2 L b2 - b3   (result overwrites z1 slot)
        cc1 = pair_barrier(1)

        def seeds2A(tt):
            return [(ident_sb[:], zslot(tt, 1))]

        def seeds2B(tt):
            return [(ident_sb[:], zslot(tt, 1)),
                    (nident_sb[:], zslot(tt, 3))]

        def finish2A(tt, ps):
            drain(tt, 1, ps)

        def finish2B(tt, ps):
            drain(tt, 1, ps)
            if tt == nt - WGRP // 2 - 1:
                write_half(2, 1, tt // WGRP, WGRP // 2)
            elif tt == nt - 1:
                g0 = nt - WGRP // 2
                w = nc.scalar.dma_start(
                    bsh[2][:, bass.ds(off_by_eng["act"] + g0, WGRP // 2), :],
                    z_sb[:, g0:g0 + WGRP // 2, 1, :, :]
                    .rearrange("p t b o -> p t (b o)"))
                shared_writes[2].append(w)
            elif (tt + 1) % WGRP == 0:
                write_half(2, 1, tt // WGRP)

        spmm_phase(bsh[1], gval2_sb, cc1, seeds2A, finish2A, seeds2B, finish2B)

        # phase 3: out = (z0 + L_own b1) + L_peer b1 - b2   (bias already in z0)
        cc2 = pair_barrier(2)

        def seeds3A(tt):
            return [(ident_sb[:], zslot(tt, 0))]

        def seeds3B(tt):
            return [(ident_sb[:], zslot(tt, 0)),
                    (nident_sb[:], zslot(tt, 2))]

        def finish3A(tt, ps):
            drain(tt, 0, ps)

        ostate = {"ot": None}

        def finish3B(tt, ps):
            if tt % WGRP == 0:
                ot_new = opool.tile([P, WGRP, F], BF16, tag="ot")
                ostate["ot"] = ot_new
            ot = ostate["ot"]
            nc.scalar.activation(out=ot[:, tt % WGRP, :], in_=ps[:],
                                 func=mybir.ActivationFunctionType.Copy)
            if (tt + 1) % WGRP == 0:
                g0 = (tt // WGRP) * WGRP
                nc.sync.dma_start(out_d[:, g0:g0 + WGRP, :], ot[:])

        spmm_phase(bsh[2], gval1_sb, cc2, seeds3A, finish3A, seeds3B, finish3B)

    nc.compile()
    return nc


def make_host_inputs(inputs, weight, bias, lap_vals, lap_rows, lap_cols):
    per_parity, chunks = _preprocess_lap(
        np.asarray(lap_rows), np.asarray(lap_cols),
        np.asarray(lap_vals, np.float32))
    w = np.asarray(weight, np.float32)
    # wz[(t,f) split cc, (k,o)]
    wz = np.transpose(w, (2, 0, 1, 3)).reshape(C, KV * FOUT)
    wz = np.ascontiguousarray(
        wz.reshape(2, P, KV * FOUT).transpose(1, 0, 2)).astype(ml_dtypes.bfloat16)
    biasw = np.zeros((1, KV * FOUT), np.float32)
    biasw[0, :FOUT] = np.asarray(bias, np.float32)
    biasw = biasw.astype(ml_dtypes.bfloat16)
    onesb = np.ones((1, P), ml_dtypes.bfloat16)
    ident128 = np.eye(P, dtype=np.float32).astype(ml_dtypes.bfloat16)
    iota128 = np.ascontiguousarray(
        np.broadcast_to(np.arange(P, dtype=np.float32)[None, :],
                        (P, P))).astype(ml_dtypes.bfloat16)
    x = np.asarray(inputs, np.float32)
    in_maps = []
    for r in range(N_CORES):
        pair, h = r // 2, r % 2
        gidx_w, growl_m, gval_m = per_parity[h]
        # xt[b, cc, cl, v] = x[4p+b, h*VH + v, t, f], c=(t,f)=cc*128+cl
        xs = x[BG * pair:BG * (pair + 1), h * VH:(h + 1) * VH]  # [4, VH, T, FIN]
        xt = xs.reshape(BG, VH, C).transpose(0, 2, 1).reshape(BG, 2, P, VH)
        m = {
            "xt": np.ascontiguousarray(xt).astype(ml_dtypes.bfloat16),
            "wz": wz,
            "biasw": biasw,
            "onesb": onesb,
            "iota128": iota128,
            "ident128": ident128,
            "nident128": -ident128,
            "offt": np.array([[h * NT]], np.int32),
            "gidx": gidx_w,
            "growl": growl_m,
            "gval1": gval_m,
            "gval2": np.ascontiguousarray(2.0 * gval_m),
        }
        in_maps.append(m)
    return in_maps, chunks


_CACHE = {}


def _get_program(chunks, has_bias):
    key = (tuple(chunks[0]), tuple(chunks[1]), has_bias)
    if key not in _CACHE:
        _CACHE[key] = build_program((list(chunks[0]), list(chunks[1])), has_bias)
    return _CACHE[key]


def kernel(inputs, weight, bias, lap_vals, lap_rows, lap_cols):
    in_maps, chunks = make_host_inputs(inputs, weight, bias, lap_vals,
                                       lap_rows, lap_cols)
    nc = _get_program(chunks, bool(np.any(np.asarray(bias))))
    res = run_bass_kernel_spmd(nc, in_maps, list(range(N_CORES)))
    out = np.empty((B, V, FOUT), np.float32)
    for r in range(N_CORES):
        pair, h = r // 2, r % 2
        o = np.asarray(res.results[r]["out"], np.float32).reshape(P, NT, BG, FOUT)
        o = o.transpose(2, 1, 0, 3).reshape(BG, VH, FOUT)
        out[BG * pair:BG * (pair + 1), h * VH:(h + 1) * VH, :] = o
    return np.ascontiguousarray(out)


def time_kernel(inputs_dict, iters=3):
    """Wall-clock repeated executions of the cached program (ns per run)."""
    import time

    in_maps, chunks = make_host_inputs(**inputs_dict)
    nc = _get_program(chunks, bool(np.any(np.asarray(inputs_dict["bias"]))))
    times = []
    for _ in range(iters):
        t0 = time.perf_counter()
        run_bass_kernel_spmd(nc, in_maps, list(range(N_CORES)))
        times.append(time.perf_counter() - t0)
    return min(times) * 1e9


# revision 50
# speedup vs baseline: 1.0152x; 1.0126x over previous
"""Trainium2 Bass kernel for ConvChebTemp (Chebyshev graph conv, temporal weights).

Math: out[b,v,o] = sum_{k,t,f} T_k(L)x0[:,t,f,b] w[f,k,t,o] + bias[o]
with x0 = inputs permuted to [V, T*Fin*B] and T_k the Chebyshev recurrence.

Clenshaw reformulation (weights contracted first):
  z_k[v,b,o] = sum_{t,f} x0[v,t,f,b] w[f,k,t,o]
  b3 = z3; b2 = z2 + 2 L b3; b1 = z1 + 2 L b2 - b3; out = z0 + L b1 - b2 + bias

Sharding: 8 cores = 4 pairs. Pair p owns batches [4p, 4p+4); within the pair
the graph rows are split in half (core 2p: rows [0, V/2), core 2p+1 the rest).
The Clenshaw iterates b3/b2/b1 live in pair-SHARED HBM tensors
(addr_space="Shared": cores (2k, 2k+1) see one physical buffer), so each
core writes only its half and gathers from the full tensor. Cross-core
ordering is a tiny per-pair AllGather barrier before each phase's gathers.

Everything on the SpMM path is bf16: gather rows are 4 batches x 64 Fout x 2B
= 512B (full DMA descriptor efficiency) and all matmuls run at 1 cycle/row.
"""
import sys

sys.path.insert(0, "/opt/trn_rl_repo")

from contextlib import ExitStack  # noqa: E402

import ml_dtypes  # noqa: E402
import numpy as np  # noqa: E402

from concourse import bacc, bass, mybir, tile  # noqa: E402
from concourse.bass_utils import run_bass_kernel_spmd  # noqa: E402

P = 128
N_CORES = 8
FP32 = mybir.dt.float32
BF16 = mybir.dt.bfloat16
I32 = mybir.dt.int32
I16 = mybir.dt.int16

# Problem dims (hardcoded per spec)
B, V, T, FIN = 16, 12288, 4, 64
KV, KT, FOUT = 4, 4, 64
VH = V // 2                # rows per core
NT = VH // P               # out-tiles per core (48)
BG = 4                     # batches per pair
F = BG * FOUT              # spmm row width (256 bf16 = 512B)
C = T * FIN                # z contraction dim (256)
PAIR_GROUPS = [[0, 1], [2, 3], [4, 5], [6, 7]]
CHUNKS_PER_PIECE = 8       # 1024 gather indices per instruction
DMA_SCRATCH = 16384        # SWDGE ring: 1024 descriptors
WGRP = 8                   # out-tiles per batched shared-HBM write


def _preprocess_lap(lap_rows, lap_cols, lap_vals):
    """Split nnz by row-half into own-column (section A) and peer-column
    (section B) chunk streams, padded to a common per-tile chunk structure
    (identical across cores so one SPMD program serves all).

    Section A only references rows this core wrote itself, so its gathers
    need no cross-core barrier.

    Returns (per_parity list of (gidx_wrapped, growl, gval),
    (chunksA_per_tile, chunksB_per_tile)).
    """
    halves = []
    cnt = np.zeros((2, 2, NT), np.int64)  # [section, parity, tile]
    for h in (0, 1):
        lo, hi = h * VH, (h + 1) * VH
        m = (lap_rows >= lo) & (lap_rows < hi)
        lrows = lap_rows[m] - lo
        order = np.argsort(lrows, kind="stable")
        lrows = lrows[order]
        cols = lap_cols[m][order]
        vals = lap_vals[m][order]
        own = (cols >= lo) & (cols < hi)
        tiles = lrows // P
        np.add.at(cnt[0, h], tiles[own], 1)
        np.add.at(cnt[1, h], tiles[~own], 1)
        halves.append((lrows, cols, vals, own, tiles))
    # pass A gets only FULL chunks of own-column nnz (min across parities so
    # neither pads); leftovers ride in pass B's first chunk, which is gathered
    # after the barrier anyway. This keeps total chunks near the unsplit count.
    chunksA = [min(int(cnt[0, 0][t] // P), int(cnt[0, 1][t] // P))
               for t in range(NT)]
    chunksB = [max(1,
                   int(-(-(cnt[0, 0][t] - P * chunksA[t] + cnt[1, 0][t]) // P)),
                   int(-(-(cnt[0, 1][t] - P * chunksA[t] + cnt[1, 1][t]) // P)))
               for t in range(NT)]
    nchunk = sum(chunksA) + sum(chunksB)
    nnzp = nchunk * P
    out = []
    for h, (lrows, cols, vals, own, tiles) in enumerate(halves):
        # pad slots must gather an own-half row (peer half may be unwritten
        # while section A streams): local row 0 of my half
        pad_v = h * VH
        gidx = np.full(nnzp, pad_v, np.int32)
        growl = np.zeros(nnzp, np.float32)
        gval = np.zeros(nnzp, np.float32)
        pos = 0
        for t in range(NT):  # section A: first P*chunksA[t] own nnz
            m = own & (tiles == t)
            n = P * chunksA[t]
            idx = np.flatnonzero(m)[:n]
            assert len(idx) == n
            gidx[pos:pos + n] = cols[idx]
            growl[pos:pos + n] = (lrows[idx] - t * P).astype(np.float32)
            gval[pos:pos + n] = vals[idx]
            pos += n
        for t in range(NT):  # section B: leftover own + all peer nnz
            m = own & (tiles == t)
            skip = P * chunksA[t]
            idx = np.concatenate([np.flatnonzero(m)[skip:],
                                  np.flatnonzero((~own) & (tiles == t))])
            n = len(idx)
            gidx[pos:pos + n] = cols[idx]
            growl[pos:pos + n] = (lrows[idx] - t * P).astype(np.float32)
            gval[pos:pos + n] = vals[idx]
            pos += chunksB[t] * P
        assert pos == nnzp
        # remap to partition-major rows: v -> (v % 128) * 96 + v // 128
        gidx = ((gidx % P) * (V // P) + gidx // P).astype(np.int16)
        gidx_w = np.tile(gidx.reshape(-1, 16).T.copy(), (8, 1))  # [128, nnzp/16]
        growl_m = growl.reshape(nchunk, P).T.copy()
        gval_m = gval.reshape(nchunk, P).T.copy()
        out.append((np.ascontiguousarray(gidx_w),
                    np.ascontiguousarray(growl_m),
                    np.ascontiguousarray(gval_m)))
    return out, (chunksA, chunksB)


def build_program(chunks_per_tile, has_bias, n_cores=N_CORES):
    nt = NT
    nchunk = sum(chunks_per_tile[0]) + sum(chunks_per_tile[1])
    nnzp = nchunk * P
    nc = bacc.Bacc("TRN2", target_bir_lowering=False, debug=False,
                   num_devices=n_cores, dynamic_dma_scratch_size=DMA_SCRATCH)

    xt_d = nc.dram_tensor("xt", [BG, 2, P, VH], BF16, kind="ExternalInput")
    wz_d = nc.dram_tensor("wz", [P, 2, KV * FOUT], BF16, kind="ExternalInput")
    onesb_d = nc.dram_tensor("onesb", [1, P], BF16, kind="ExternalInput")
    biasw_d = nc.dram_tensor("biasw", [1, KV * FOUT], BF16, kind="ExternalInput")
    iota_d = nc.dram_tensor("iota128", [P, P], BF16, kind="ExternalInput")
    ident_d = nc.dram_tensor("ident128", [P, P], BF16, kind="ExternalInput")
    nident_d = nc.dram_tensor("nident128", [P, P], BF16, kind="ExternalInput")
    offt_d = nc.dram_tensor("offt", [1, 1], I32, kind="ExternalInput")
    gidx_d = nc.dram_tensor("gidx", [P, nnzp // 16], I16, kind="ExternalInput")
    growl_d = nc.dram_tensor("growl", [P, nchunk], FP32, kind="ExternalInput")
    gval1_d = nc.dram_tensor("gval1", [P, nchunk], FP32, kind="ExternalInput")
    gval2_d = nc.dram_tensor("gval2", [P, nchunk], FP32, kind="ExternalInput")
    out_d = nc.dram_tensor("out", [P, NT, F], BF16, kind="ExternalOutput")

    # pair-shared Clenshaw iterates (both cores of a pair see one buffer),
    # stored partition-major: row v lives at [v % 128, v // 128, :] so the
    # per-core half writes are 128 contiguous 4KB descriptors per group
    bsh = [nc.dram_tensor(f"bsh{k}", [P, V // P, F], BF16, kind="Internal",
                          addr_space="Shared") for k in range(3)]
    bin_d = [nc.dram_tensor(f"bin{k}", [1, 16], BF16, kind="Internal")
             for k in range(3)]
    bout_d = [nc.dram_tensor(f"bout{k}", [2, 16], BF16, kind="Internal")
              for k in range(3)]

    with tile.TileContext(nc) as tc, ExitStack() as ctx:
        const = ctx.enter_context(tc.tile_pool(name="const", bufs=1))
        zres = ctx.enter_context(tc.tile_pool(name="zres", bufs=1))
        xpool = ctx.enter_context(tc.tile_pool(name="x", bufs=2))
        gpool = ctx.enter_context(tc.tile_pool(name="gbuf", bufs=5))
        spool = ctx.enter_context(tc.tile_pool(name="sel", bufs=4))
        opool = ctx.enter_context(tc.tile_pool(name="ostg", bufs=2))
        bpool = ctx.enter_context(tc.tile_pool(name="bounce", bufs=1))
        psz = ctx.enter_context(tc.tile_pool(name="psz", bufs=4, space="PSUM"))
        pss = ctx.enter_context(tc.tile_pool(name="pss", bufs=4, space="PSUM"))

        # constants + metadata resident in SBUF
        iota_sb = const.tile([P, P], BF16, tag="iota")
        nc.sync.dma_start(iota_sb[:], iota_d[:, :])
        ident_sb = const.tile([P, P], BF16, tag="ident")
        nc.sync.dma_start(ident_sb[:], ident_d[:, :])
        nident_sb = const.tile([P, P], BF16, tag="nident")
        nc.sync.dma_start(nident_sb[:], nident_d[:, :])
        ones_sb = const.tile([1, P], BF16, tag="ones")
        nc.sync.dma_start(ones_sb[:], onesb_d[:, :])
        biasw_sb = const.tile([1, KV * FOUT], BF16, tag="biasw")
        nc.sync.dma_start(biasw_sb[:], biasw_d[:, :])
        wz_sb = const.tile([P, 2, KV * FOUT], BF16, tag="wz")
        nc.sync.dma_start(wz_sb[:], wz_d[:, :, :])
        gidx_sb = const.tile([P, nnzp // 16], I16, tag="gidx")
        nc.sync.dma_start(gidx_sb[:], gidx_d[:, :])
        growl_sb = const.tile([P, nchunk], FP32, tag="growl")
        nc.sync.dma_start(growl_sb[:], growl_d[:, :])
        gval1_sb = const.tile([P, nchunk], FP32, tag="gval1")
        nc.sync.dma_start(gval1_sb[:], gval1_d[:, :])
        gval2_sb = const.tile([P, nchunk], FP32, tag="gval2")
        nc.sync.dma_start(gval2_sb[:], gval2_d[:, :])

        # my tile offset into the shared tensors (0 or NT); loaded on both
        # engines that issue symbolic shared writes (SP for the Z-phase b3
        # writes, Activation for the spmm-phase writes)
        off_by_eng = {}
        for eng, nm in ((nc.scalar, "act"), (nc.sync, "sp")):
            off_reg = eng.alloc_register(f"slab_off_{nm}")
            eng.reg_load(off_reg, offt_d[0:1, 0:1])
            off_by_eng[nm] = eng.snap(off_reg, donate=True, min_val=0,
                                      max_val=NT)

        # all z_k resident in SBUF: [P, nt, KV, BG, FOUT] bf16 (96KB/partition)
        z_sb = zres.tile([P, nt, KV, BG, FOUT], BF16, tag="z")

        shared_writes = {0: [], 1: [], 2: []}

        def write_half(kidx, kslot, grp, ntiles=WGRP):
            """Batched write of ntiles tiles of z-slot kslot to shared bsh[kidx]."""
            g0 = grp * WGRP
            eng, off = ((nc.sync, "sp") if kidx == 0 else (nc.scalar, "act"))
            dst = bsh[kidx][:, bass.ds(off_by_eng[off] + g0, ntiles), :]
            src = z_sb[:, g0:g0 + ntiles, kslot, :, :] \
                .rearrange("p t b o -> p t (b o)")
            w = eng.dma_start(dst, src)
            shared_writes[kidx].append(w)

        # ---------- phase Z: z_k = x0 @ w_k (+ bias folded into z0) ----------
        VHH = VH // 2
        for b in range(BG):
          for half in range(2):
            v0 = half * VHH
            xb = xpool.tile([P, 2, VHH], BF16, tag="xb")
            nc.sync.dma_start(
                xb[:], xt_d[b, :, :, v0:v0 + VHH].rearrange("c p v -> p c v"))
            for vt0 in range(half * nt // 2, (half + 1) * nt // 2, 2):
                zps = psz.tile([P, 2, KV * FOUT], FP32, tag="zps")
                for sub in range(2):
                    vt = vt0 + sub
                    for cc in range(2):
                        nc.tensor.matmul(
                            zps[:, sub, :],
                            lhsT=xb[:, cc, vt * P - v0:(vt + 1) * P - v0],
                            rhs=wz_sb[:, cc, :],
                            start=(cc == 0),
                            stop=(cc == 1 and not has_bias))
                    if has_bias:
                        nc.tensor.matmul(zps[:, sub, :], lhsT=ones_sb[:, :],
                                         rhs=biasw_sb[:, :], start=False,
                                         stop=True)
                # PSUM->SBUF cast copies: DVE 1/3, Act 2/3 (Act is cheaper)
                if (vt0 // 2) % 3 == 0:
                    nc.vector.tensor_copy(
                        z_sb[:, vt0:vt0 + 2, :, b, :],
                        zps[:].rearrange("p s (k o) -> p s k o", o=FOUT))
                else:
                    nc.scalar.activation(
                        out=z_sb[:, vt0:vt0 + 2, :, b, :],
                        in_=zps[:].rearrange("p s (k o) -> p s k o", o=FOUT),
                        func=mybir.ActivationFunctionType.Copy)
                if b == BG - 1 and (vt0 + 2) % WGRP == 0:
                    write_half(0, 3, vt0 // WGRP)

        def pair_barrier(k):
            # the AllGather is a pure rendezvous: gate it on ALL my shared
            # writes; completion proves the peer's writes are done too (the
            # payload itself is never read)
            cc = nc.gpsimd.collective_compute(
                "AllGather", mybir.AluOpType.bypass, PAIR_GROUPS,
                ins=[bin_d[k][0:1, :]], outs=[bout_d[k][:, :]])
            for w in shared_writes[k]:
                bass._add_dep_helper(cc.ins, w.ins, sync=True,
                                     reason="barrier after all shared writes")
            return cc

        # ---------- spmm phases ----------
        # Each phase runs in two passes: pass A covers own-half columns
        # (rows this core wrote -> no cross-core barrier; overlaps with the
        # AllGather rendezvous), pass B covers peer-half columns and waits
        # on the barrier. z-slots accumulate partials between the passes.
        chunksA, chunksB = chunks_per_tile
        nA = sum(chunksA)

        def spmm_pass(src_d, vals_sb, base0, chunks_list, nend, dep_inst,
                      seeds, finish):
            state = {"gb": None, "base": base0, "len": 0}

            def ensure_piece(c):
                while state["gb"] is None or c >= state["base"] + state["len"]:
                    base = (base0 if state["gb"] is None
                            else state["base"] + state["len"])
                    plen = min(CHUNKS_PER_PIECE, nend - base)
                    gb = gpool.tile([P, plen, F], BF16, tag="gb")
                    s0 = base * P
                    nidx = plen * P
                    g = nc.gpsimd.dma_gather(
                        out_ap=gb[:],
                        in_ap=src_d[:, :, :].rearrange("p t f -> (p t) f"),
                        idxs_ap=gidx_sb[:, s0 // 16:(s0 + nidx) // 16],
                        num_idxs=nidx,
                        num_idxs_reg=nidx,
                        elem_size=F,
                    )
                    if dep_inst is not None:
                        bass._add_dep_helper(g.ins, dep_inst.ins, sync=True,
                                             reason="pair barrier before gather")
                    state.update(gb=gb, base=base, len=plen)
                return state["gb"], state["base"]

            ci = base0
            for tt in range(nt):
                nck = chunks_list[tt]
                if nck == 0:
                    continue
                ps = pss.tile([P, F], FP32, tag="ps")
                # seed the accumulator with the running z-slot value(s) so
                # the DVE never has to do the adds
                sds = seeds(tt)
                for si, (w, src) in enumerate(sds):
                    nc.tensor.matmul(ps[:], lhsT=w, rhs=src,
                                     start=(si == 0), stop=False)
                for k in range(nck):
                    col = ci + k
                    gb, base = ensure_piece(col)
                    sT = spool.tile([P, P], BF16, tag="sT")
                    nc.vector.tensor_scalar(
                        out=sT[:], in0=iota_sb[:],
                        scalar1=growl_sb[:, col:col + 1],
                        scalar2=vals_sb[:, col:col + 1],
                        op0=mybir.AluOpType.is_equal,
                        op1=mybir.AluOpType.mult,
                    )
                    nc.tensor.matmul(ps[:], lhsT=sT[:], rhs=gb[:, col - base, :],
                                     start=False, stop=(k == nck - 1))
                finish(tt, ps)
                ci += nck

        def spmm_phase(src_d, vals_sb, cc_inst, seedsA, finishA, seedsB,
                       finishB):
            spmm_pass(src_d, vals_sb, 0, chunksA, nA, None, seedsA, finishA)
            spmm_pass(src_d, vals_sb, nA, chunksB, nchunk, cc_inst, seedsB,
                      finishB)

        def zslot(vt, k):
            return z_sb[:, vt, k, :, :].rearrange("p b o -> p (b o)")

        def ps3(ps):
            return ps[:].rearrange("p (b o) -> p b o", o=FOUT)

        def drain(tt, k, ps):
            nc.scalar.activation(out=zslot(tt, k), in_=ps[:],
                                 func=mybir.ActivationFunctionType.Copy)

        # phase 1: b2 = z2 + 2 L b3   (result overwrites z2 slot)
        cc0 = pair_barrier(0)

        def seeds1(tt):
            return [(ident_sb[:], zslot(tt, 2))]

        def finish1A(tt, ps):
            drain(tt, 2, ps)

        def finish1B(tt, ps):
            drain(tt, 2, ps)
            if tt == nt - WGRP // 2 - 1:
                write_half(1, 2, tt // WGRP, WGRP // 2)
            elif tt == nt - 1:
                g0 = nt - WGRP // 2
                w = nc.scalar.dma_start(
                    bsh[1][:, bass.ds(off_by_eng["act"] + g0, WGRP // 2), :],
                    z_sb[:, g0:g0 + WGRP // 2, 2, :, :]
                    .rearrange("p t b o -> p t (b o)"))
                shared_writes[1].append(w)
            elif (tt + 1) % WGRP == 0:
                write_half(1, 2, tt // WGRP)

        spmm_phase(bsh[0], gval2_sb, cc0, seeds1, finish1A, seeds1, finish1B)

        # phase 2: b1 = z1 + 2 L b2 - b3   (result overwrites z1 slot)
        cc1 = pair_barrier(1)

        def seeds2A(tt):
            return [(ident_sb[:], zslot(tt, 1))]

        def seeds2B(tt):
            return [(ident_sb[:], zslot(tt, 1)),
                    (nident_sb[:], zslot(tt, 3))]

        def finish2A(tt, ps):
            drain(tt, 1, ps)

        def finish2B(tt, ps):
            drain(tt, 1, ps)
            if tt == nt - WGRP // 2 - 1:
                write_half(2, 1, tt // WGRP, WGRP // 2)
            elif tt == nt - 1:
                g0 = nt - WGRP // 2
                w = nc.scalar.dma_start(
                    bsh[2][:, bass.ds(off_by_eng["act"] + g0, WGRP // 2), :],
                    z_sb[:, g0:g0 + WGRP // 2, 1, :, :]
                    .rearrange("p t b o -> p t (b o)"))
                shared_writes[2].append(w)
            elif (tt + 1) % WGRP == 0:
                write_half(2, 1, tt // WGRP)

        spmm_phase(bsh[1], gval2_sb, cc1, seeds2A, finish2A, seeds2B, finish2B)

        # phase 3: out = (z0 + L_own b1) + L_peer b1 - b2   (bias already in z0)
        cc2 = pair_barrier(2)

        def seeds3A(tt):
            return [(ident_sb[:], zslot(tt, 0))]

        def seeds3B(tt):
            return [(ident_sb[:], zslot(tt, 0)),
                    (nident_sb[:], zslot(tt, 2))]

        def finish3A(tt, ps):
            drain(tt, 0, ps)

        ostate = {"ot": None}

        def finish3B(tt, ps):
            if tt % WGRP == 0:
                ot_new = opool.tile([P, WGRP, F], BF16, tag="ot")
                ostate["ot"] = ot_new
            ot = ostate["ot"]
            nc.scalar.activation(out=ot[:, tt % WGRP, :], in_=ps[:],
                                 func=mybir.ActivationFunctionType.Copy)
            if (tt + 1) % WGRP == 0:
                g0 = (tt // WGRP) * WGRP
                nc.sync.dma_start(out_d[:, g0:g0 + WGRP, :], ot[:])

        spmm_phase(bsh[2], gval1_sb, cc2, seeds3A, finish3A, seeds3B, finish3B)

    nc.compile()
    return nc


def make_host_inputs(inputs, weight, bias, lap_vals, lap_rows, lap_cols):
    per_parity, chunks = _preprocess_lap(
        np.asarray(lap_rows), np.asarray(lap_cols),
        np.asarray(lap_vals, np.float32))
    w = np.asarray(weight, np.float32)
    # wz[(t,f) split cc, (k,o)]
    wz = np.transpose(w, (2, 0, 1, 3)).reshape(C, KV * FOUT)
    wz = np.ascontiguousarray(
        wz.reshape(2, P, KV * FOUT).transpose(1, 0, 2)).astype(ml_dtypes.bfloat16)
    biasw = np.zeros((1, KV * FOUT), np.float32)
    biasw[0, :FOUT] = np.asarray(bias, np.float32)
    biasw = biasw.astype(ml_dtypes.bfloat16)
    onesb = np.ones((1, P), ml_dtypes.bfloat16)
    ident128 = np.eye(P, dtype=np.float32).astype(ml_dtypes.bfloat16)
    iota128 = np.ascontiguousarray(
        np.broadcast_to(np.arange(P, dtype=np.float32)[None, :],
                        (P, P))).astype(ml_dtypes.bfloat16)
    x = np.asarray(inputs, np.float32)
    in_maps = []
    for r in range(N_CORES):
        pair, h = r // 2, r % 2
        gidx_w, growl_m, gval_m = per_parity[h]
        # xt[b, cc, cl, v] = x[4p+b, h*VH + v, t, f], c=(t,f)=cc*128+cl
        xs = x[BG * pair:BG * (pair + 1), h * VH:(h + 1) * VH]  # [4, VH, T, FIN]
        xt = xs.reshape(BG, VH, C).transpose(0, 2, 1).reshape(BG, 2, P, VH)
        m = {
            "xt": np.ascontiguousarray(xt).astype(ml_dtypes.bfloat16),
            "wz": wz,
            "biasw": biasw,
            "onesb": onesb,
            "iota128": iota128,
            "ident128": ident128,
            "nident128": -ident128,
            "offt": np.array([[h * NT]], np.int32),
            "gidx": gidx_w,
            "growl": growl_m,
            "gval1": gval_m,
            "gval2": np.ascontiguousarray(2.0 * gval_m),
        }
        in_maps.append(m)
    return in_maps, chunks


_CACHE = {}


def _get_program(chunks, has_bias):
    key = (tuple(chunks[0]), tuple(chunks[1]), has_bias)
    if key not in _CACHE:
        _CACHE[key] = build_program((list(chunks[0]), list(chunks[1])), has_bias)
    return _CACHE[key]


def kernel(inputs, weight, bias, lap_vals, lap_rows, lap_cols):
    in_maps, chunks = make_host_inputs(inputs, weight, bias, lap_vals,
                                       lap_rows, lap_cols)
    nc = _get_program(chunks, bool(np.any(np.asarray(bias))))
    res = run_bass_kernel_spmd(nc, in_maps, list(range(N_CORES)))
    out = np.empty((B, V, FOUT), np.float32)
    for r in range(N_CORES):
        pair, h = r // 2, r % 2
        o = np.asarray(res.results[r]["out"], np.float32).reshape(P, NT, BG, FOUT)
        o = o.transpose(2, 1, 0, 3).reshape(BG, VH, FOUT)
        out[BG * pair:BG * (pair + 1), h * VH:(h + 1) * VH, :] = o
    return np.ascontiguousarray(out)


def time_kernel(inputs_dict, iters=3):
    """Wall-clock repeated executions of the cached program (ns per run)."""
    import time

    in_maps, chunks = make_host_inputs(**inputs_dict)
    nc = _get_program(chunks, bool(np.any(np.asarray(inputs_dict["bias"]))))
    times = []
    for _ in range(iters):
        t0 = time.perf_counter()
        run_bass_kernel_spmd(nc, in_maps, list(range(N_CORES)))
        times.append(time.perf_counter() - t0)
    return min(times) * 1e9


# revision 51
# speedup vs baseline: 1.0189x; 1.0036x over previous
"""Trainium2 Bass kernel for ConvChebTemp (Chebyshev graph conv, temporal weights).

Math: out[b,v,o] = sum_{k,t,f} T_k(L)x0[:,t,f,b] w[f,k,t,o] + bias[o]
with x0 = inputs permuted to [V, T*Fin*B] and T_k the Chebyshev recurrence.

Clenshaw reformulation (weights contracted first):
  z_k[v,b,o] = sum_{t,f} x0[v,t,f,b] w[f,k,t,o]
  b3 = z3; b2 = z2 + 2 L b3; b1 = z1 + 2 L b2 - b3; out = z0 + L b1 - b2 + bias

Sharding: 8 cores = 4 pairs. Pair p owns batches [4p, 4p+4); within the pair
the graph rows are split in half (core 2p: rows [0, V/2), core 2p+1 the rest).
The Clenshaw iterates b3/b2/b1 live in pair-SHARED HBM tensors
(addr_space="Shared": cores (2k, 2k+1) see one physical buffer), so each
core writes only its half and gathers from the full tensor. Cross-core
ordering is a tiny per-pair AllGather barrier before each phase's gathers.

Everything on the SpMM path is bf16: gather rows are 4 batches x 64 Fout x 2B
= 512B (full DMA descriptor efficiency) and all matmuls run at 1 cycle/row.
"""
import sys

sys.path.insert(0, "/opt/trn_rl_repo")

from contextlib import ExitStack  # noqa: E402

import ml_dtypes  # noqa: E402
import numpy as np  # noqa: E402

from concourse import bacc, bass, mybir, tile  # noqa: E402
from concourse.bass_utils import run_bass_kernel_spmd  # noqa: E402

P = 128
N_CORES = 8
FP32 = mybir.dt.float32
BF16 = mybir.dt.bfloat16
I32 = mybir.dt.int32
I16 = mybir.dt.int16

# Problem dims (hardcoded per spec)
B, V, T, FIN = 16, 12288, 4, 64
KV, KT, FOUT = 4, 4, 64
VH = V // 2                # rows per core
NT = VH // P               # out-tiles per core (48)
BG = 4                     # batches per pair
F = BG * FOUT              # spmm row width (256 bf16 = 512B)
C = T * FIN                # z contraction dim (256)
PAIR_GROUPS = [[0, 1], [2, 3], [4, 5], [6, 7]]
CHUNKS_PER_PIECE = 8       # 1024 gather indices per instruction
DMA_SCRATCH = 16384        # SWDGE ring: 1024 descriptors
WGRP = 8                   # out-tiles per batched shared-HBM write


def _preprocess_lap(lap_rows, lap_cols, lap_vals):
    """Split nnz by row-half into own-column (section A) and peer-column
    (section B) chunk streams, padded to a common per-tile chunk structure
    (identical across cores so one SPMD program serves all).

    Section A only references rows this core wrote itself, so its gathers
    need no cross-core barrier.

    Returns (per_parity list of (gidx_wrapped, growl, gval),
    (chunksA_per_tile, chunksB_per_tile)).
    """
    halves = []
    cnt = np.zeros((2, 2, NT), np.int64)  # [section, parity, tile]
    for h in (0, 1):
        lo, hi = h * VH, (h + 1) * VH
        m = (lap_rows >= lo) & (lap_rows < hi)
        lrows = lap_rows[m] - lo
        order = np.argsort(lrows, kind="stable")
        lrows = lrows[order]
        cols = lap_cols[m][order]
        vals = lap_vals[m][order]
        own = (cols >= lo) & (cols < hi)
        tiles = lrows // P
        np.add.at(cnt[0, h], tiles[own], 1)
        np.add.at(cnt[1, h], tiles[~own], 1)
        halves.append((lrows, cols, vals, own, tiles))
    # pass A gets only FULL chunks of own-column nnz (min across parities so
    # neither pads); leftovers ride in pass B's first chunk, which is gathered
    # after the barrier anyway. This keeps total chunks near the unsplit count.
    chunksA = [min(int(cnt[0, 0][t] // P), int(cnt[0, 1][t] // P))
               for t in range(NT)]
    chunksB = [max(1,
                   int(-(-(cnt[0, 0][t] - P * chunksA[t] + cnt[1, 0][t]) // P)),
                   int(-(-(cnt[0, 1][t] - P * chunksA[t] + cnt[1, 1][t]) // P)))
               for t in range(NT)]
    nchunk = sum(chunksA) + sum(chunksB)
    nnzp = nchunk * P
    out = []
    for h, (lrows, cols, vals, own, tiles) in enumerate(halves):
        # pad slots must gather an own-half row (peer half may be unwritten
        # while section A streams): local row 0 of my half
        pad_v = h * VH
        gidx = np.full(nnzp, pad_v, np.int32)
        growl = np.zeros(nnzp, np.float32)
        gval = np.zeros(nnzp, np.float32)
        pos = 0
        for t in range(NT):  # section A: first P*chunksA[t] own nnz
            m = own & (tiles == t)
            n = P * chunksA[t]
            idx = np.flatnonzero(m)[:n]
            assert len(idx) == n
            gidx[pos:pos + n] = cols[idx]
            growl[pos:pos + n] = (lrows[idx] - t * P).astype(np.float32)
            gval[pos:pos + n] = vals[idx]
            pos += n
        for t in range(NT):  # section B: leftover own + all peer nnz
            m = own & (tiles == t)
            skip = P * chunksA[t]
            idx = np.concatenate([np.flatnonzero(m)[skip:],
                                  np.flatnonzero((~own) & (tiles == t))])
            n = len(idx)
            gidx[pos:pos + n] = cols[idx]
            growl[pos:pos + n] = (lrows[idx] - t * P).astype(np.float32)
            gval[pos:pos + n] = vals[idx]
            pos += chunksB[t] * P
        assert pos == nnzp
        # remap to partition-major rows: v -> (v % 128) * 96 + v // 128
        gidx = ((gidx % P) * (V // P) + gidx // P).astype(np.int16)
        gidx_w = np.tile(gidx.reshape(-1, 16).T.copy(), (8, 1))  # [128, nnzp/16]
        growl_m = growl.reshape(nchunk, P).T.copy()
        gval_m = gval.reshape(nchunk, P).T.copy()
        out.append((np.ascontiguousarray(gidx_w),
                    np.ascontiguousarray(growl_m),
                    np.ascontiguousarray(gval_m)))
    return out, (chunksA, chunksB)


def build_program(chunks_per_tile, has_bias, n_cores=N_CORES):
    nt = NT
    nchunk = sum(chunks_per_tile[0]) + sum(chunks_per_tile[1])
    nnzp = nchunk * P
    nc = bacc.Bacc("TRN2", target_bir_lowering=False, debug=False,
                   num_devices=n_cores, dynamic_dma_scratch_size=DMA_SCRATCH)

    xt_d = nc.dram_tensor("xt", [BG, 2, P, VH], BF16, kind="ExternalInput")
    wz_d = nc.dram_tensor("wz", [P, 2, KV * FOUT], BF16, kind="ExternalInput")
    onesb_d = nc.dram_tensor("onesb", [1, P], BF16, kind="ExternalInput")
    biasw_d = nc.dram_tensor("biasw", [1, KV * FOUT], BF16, kind="ExternalInput")
    iota_d = nc.dram_tensor("iota128", [P, P], BF16, kind="ExternalInput")
    ident_d = nc.dram_tensor("ident128", [P, P], BF16, kind="ExternalInput")
    nident_d = nc.dram_tensor("nident128", [P, P], BF16, kind="ExternalInput")
    offt_d = nc.dram_tensor("offt", [1, 1], I32, kind="ExternalInput")
    gidx_d = nc.dram_tensor("gidx", [P, nnzp // 16], I16, kind="ExternalInput")
    growl_d = nc.dram_tensor("growl", [P, nchunk], FP32, kind="ExternalInput")
    gval1_d = nc.dram_tensor("gval1", [P, nchunk], FP32, kind="ExternalInput")
    gval2_d = nc.dram_tensor("gval2", [P, nchunk], FP32, kind="ExternalInput")
    out_d = nc.dram_tensor("out", [P, NT, F], BF16, kind="ExternalOutput")

    # pair-shared Clenshaw iterates (both cores of a pair see one buffer),
    # stored partition-major: row v lives at [v % 128, v // 128, :] so the
    # per-core half writes are 128 contiguous 4KB descriptors per group
    bsh = [nc.dram_tensor(f"bsh{k}", [P, V // P, F], BF16, kind="Internal",
                          addr_space="Shared") for k in range(3)]
    bin_d = [nc.dram_tensor(f"bin{k}", [1, 16], BF16, kind="Internal")
             for k in range(3)]
    bout_d = [nc.dram_tensor(f"bout{k}", [2, 16], BF16, kind="Internal")
              for k in range(3)]

    with tile.TileContext(nc) as tc, ExitStack() as ctx:
        const = ctx.enter_context(tc.tile_pool(name="const", bufs=1))
        zres = ctx.enter_context(tc.tile_pool(name="zres", bufs=1))
        xpool = ctx.enter_context(tc.tile_pool(name="x", bufs=2))
        gpool = ctx.enter_context(tc.tile_pool(name="gbuf", bufs=6))
        spool = ctx.enter_context(tc.tile_pool(name="sel", bufs=4))
        opool = ctx.enter_context(tc.tile_pool(name="ostg", bufs=2))
        bpool = ctx.enter_context(tc.tile_pool(name="bounce", bufs=1))
        psz = ctx.enter_context(tc.tile_pool(name="psz", bufs=4, space="PSUM"))
        pss = ctx.enter_context(tc.tile_pool(name="pss", bufs=4, space="PSUM"))

        # constants + metadata resident in SBUF
        iota_sb = const.tile([P, P], BF16, tag="iota")
        nc.sync.dma_start(iota_sb[:], iota_d[:, :])
        ident_sb = const.tile([P, P], BF16, tag="ident")
        nc.sync.dma_start(ident_sb[:], ident_d[:, :])
        nident_sb = const.tile([P, P], BF16, tag="nident")
        nc.sync.dma_start(nident_sb[:], nident_d[:, :])
        ones_sb = const.tile([1, P], BF16, tag="ones")
        nc.sync.dma_start(ones_sb[:], onesb_d[:, :])
        biasw_sb = const.tile([1, KV * FOUT], BF16, tag="biasw")
        nc.sync.dma_start(biasw_sb[:], biasw_d[:, :])
        wz_sb = const.tile([P, 2, KV * FOUT], BF16, tag="wz")
        nc.sync.dma_start(wz_sb[:], wz_d[:, :, :])
        gidx_sb = const.tile([P, nnzp // 16], I16, tag="gidx")
        nc.sync.dma_start(gidx_sb[:], gidx_d[:, :])
        growl_sb = const.tile([P, nchunk], FP32, tag="growl")
        nc.sync.dma_start(growl_sb[:], growl_d[:, :])
        gval1_sb = const.tile([P, nchunk], FP32, tag="gval1")
        nc.sync.dma_start(gval1_sb[:], gval1_d[:, :])
        gval2_sb = const.tile([P, nchunk], FP32, tag="gval2")
        nc.sync.dma_start(gval2_sb[:], gval2_d[:, :])

        # my tile offset into the shared tensors (0 or NT); loaded on both
        # engines that issue symbolic shared writes (SP for the Z-phase b3
        # writes, Activation for the spmm-phase writes)
        off_by_eng = {}
        for eng, nm in ((nc.scalar, "act"), (nc.sync, "sp")):
            off_reg = eng.alloc_register(f"slab_off_{nm}")
            eng.reg_load(off_reg, offt_d[0:1, 0:1])
            off_by_eng[nm] = eng.snap(off_reg, donate=True, min_val=0,
                                      max_val=NT)

        # all z_k resident in SBUF: [P, nt, KV, BG, FOUT] bf16 (96KB/partition)
        z_sb = zres.tile([P, nt, KV, BG, FOUT], BF16, tag="z")

        shared_writes = {0: [], 1: [], 2: []}

        def write_half(kidx, kslot, grp, ntiles=WGRP):
            """Batched write of ntiles tiles of z-slot kslot to shared bsh[kidx]."""
            g0 = grp * WGRP
            eng, off = ((nc.sync, "sp") if kidx == 0 else (nc.scalar, "act"))
            dst = bsh[kidx][:, bass.ds(off_by_eng[off] + g0, ntiles), :]
            src = z_sb[:, g0:g0 + ntiles, kslot, :, :] \
                .rearrange("p t b o -> p t (b o)")
            w = eng.dma_start(dst, src)
            shared_writes[kidx].append(w)

        # ---------- phase Z: z_k = x0 @ w_k (+ bias folded into z0) ----------
        VHH = VH // 2
        for b in range(BG):
          for half in range(2):
            v0 = half * VHH
            xb = xpool.tile([P, 2, VHH], BF16, tag="xb")
            nc.sync.dma_start(
                xb[:], xt_d[b, :, :, v0:v0 + VHH].rearrange("c p v -> p c v"))
            for vt0 in range(half * nt // 2, (half + 1) * nt // 2, 2):
                zps = psz.tile([P, 2, KV * FOUT], FP32, tag="zps")
                for sub in range(2):
                    vt = vt0 + sub
                    for cc in range(2):
                        nc.tensor.matmul(
                            zps[:, sub, :],
                            lhsT=xb[:, cc, vt * P - v0:(vt + 1) * P - v0],
                            rhs=wz_sb[:, cc, :],
                            start=(cc == 0),
                            stop=(cc == 1 and not has_bias))
                    if has_bias:
                        nc.tensor.matmul(zps[:, sub, :], lhsT=ones_sb[:, :],
                                         rhs=biasw_sb[:, :], start=False,
                                         stop=True)
                # PSUM->SBUF cast copies: DVE 1/3, Act 2/3 (Act is cheaper)
                if (vt0 // 2) % 3 == 0:
                    nc.vector.tensor_copy(
                        z_sb[:, vt0:vt0 + 2, :, b, :],
                        zps[:].rearrange("p s (k o) -> p s k o", o=FOUT))
                else:
                    nc.scalar.activation(
                        out=z_sb[:, vt0:vt0 + 2, :, b, :],
                        in_=zps[:].rearrange("p s (k o) -> p s k o", o=FOUT),
                        func=mybir.ActivationFunctionType.Copy)
                if b == BG - 1 and (vt0 + 2) % WGRP == 0:
                    write_half(0, 3, vt0 // WGRP)

        def pair_barrier(k):
            # the AllGather is a pure rendezvous: gate it on ALL my shared
            # writes; completion proves the peer's writes are done too (the
            # payload itself is never read)
            cc = nc.gpsimd.collective_compute(
                "AllGather", mybir.AluOpType.bypass, PAIR_GROUPS,
                ins=[bin_d[k][0:1, :]], outs=[bout_d[k][:, :]])
            for w in shared_writes[k]:
                bass._add_dep_helper(cc.ins, w.ins, sync=True,
                                     reason="barrier after all shared writes")
            return cc

        # ---------- spmm phases ----------
        # Each phase runs in two passes: pass A covers own-half columns
        # (rows this core wrote -> no cross-core barrier; overlaps with the
        # AllGather rendezvous), pass B covers peer-half columns and waits
        # on the barrier. z-slots accumulate partials between the passes.
        chunksA, chunksB = chunks_per_tile
        nA = sum(chunksA)

        def spmm_pass(src_d, vals_sb, base0, chunks_list, nend, dep_inst,
                      seeds, finish):
            state = {"gb": None, "base": base0, "len": 0}

            def ensure_piece(c):
                while state["gb"] is None or c >= state["base"] + state["len"]:
                    base = (base0 if state["gb"] is None
                            else state["base"] + state["len"])
                    plen = min(CHUNKS_PER_PIECE, nend - base)
                    gb = gpool.tile([P, plen, F], BF16, tag="gb")
                    s0 = base * P
                    nidx = plen * P
                    g = nc.gpsimd.dma_gather(
                        out_ap=gb[:],
                        in_ap=src_d[:, :, :].rearrange("p t f -> (p t) f"),
                        idxs_ap=gidx_sb[:, s0 // 16:(s0 + nidx) // 16],
                        num_idxs=nidx,
                        num_idxs_reg=nidx,
                        elem_size=F,
                    )
                    if dep_inst is not None:
                        bass._add_dep_helper(g.ins, dep_inst.ins, sync=True,
                                             reason="pair barrier before gather")
                    state.update(gb=gb, base=base, len=plen)
                return state["gb"], state["base"]

            ci = base0
            for tt in range(nt):
                nck = chunks_list[tt]
                if nck == 0:
                    continue
                ps = pss.tile([P, F], FP32, tag="ps")
                # seed the accumulator with the running z-slot value(s) so
                # the DVE never has to do the adds
                sds = seeds(tt)
                for si, (w, src) in enumerate(sds):
                    nc.tensor.matmul(ps[:], lhsT=w, rhs=src,
                                     start=(si == 0), stop=False)
                for k in range(nck):
                    col = ci + k
                    gb, base = ensure_piece(col)
                    sT = spool.tile([P, P], BF16, tag="sT")
                    nc.vector.tensor_scalar(
                        out=sT[:], in0=iota_sb[:],
                        scalar1=growl_sb[:, col:col + 1],
                        scalar2=vals_sb[:, col:col + 1],
                        op0=mybir.AluOpType.is_equal,
                        op1=mybir.AluOpType.mult,
                    )
                    nc.tensor.matmul(ps[:], lhsT=sT[:], rhs=gb[:, col - base, :],
                                     start=False, stop=(k == nck - 1))
                finish(tt, ps)
                ci += nck

        def spmm_phase(src_d, vals_sb, cc_inst, seedsA, finishA, seedsB,
                       finishB):
            spmm_pass(src_d, vals_sb, 0, chunksA, nA, None, seedsA, finishA)
            spmm_pass(src_d, vals_sb, nA, chunksB, nchunk, cc_inst, seedsB,
                      finishB)

        def zslot(vt, k):
            return z_sb[:, vt, k, :, :].rearrange("p b o -> p (b o)")

        def ps3(ps):
            return ps[:].rearrange("p (b o) -> p b o", o=FOUT)

        def drain(tt, k, ps):
            nc.scalar.activation(out=zslot(tt, k), in_=ps[:],
                                 func=mybir.ActivationFunctionType.Copy)

        # phase 1: b2 = z2 + 2 L b3   (result overwrites z2 slot)
        cc0 = pair_barrier(0)

        def seeds1(tt):
            return [(ident_sb[:], zslot(tt, 2))]

        def finish1A(tt, ps):
            drain(tt, 2, ps)

        def finish1B(tt, ps):
            drain(tt, 2, ps)
            if tt == nt - WGRP // 2 - 1:
                write_half(1, 2, tt // WGRP, WGRP // 2)
            elif tt == nt - 1:
                g0 = nt - WGRP // 2
                w = nc.scalar.dma_start(
                    bsh[1][:, bass.ds(off_by_eng["act"] + g0, WGRP // 2), :],
                    z_sb[:, g0:g0 + WGRP // 2, 2, :, :]
                    .rearrange("p t b o -> p t (b o)"))
                shared_writes[1].append(w)
            elif (tt + 1) % WGRP == 0:
                write_half(1, 2, tt // WGRP)

        spmm_phase(bsh[0], gval2_sb, cc0, seeds1, finish1A, seeds1, finish1B)

        # phase 2: b1 = z1 + 2 L b2 - b3   (result overwrites z1 slot)
        cc1 = pair_barrier(1)

        def seeds2A(tt):
            return [(ident_sb[:], zslot(tt, 1))]

        def seeds2B(tt):
            return [(ident_sb[:], zslot(tt, 1)),
                    (nident_sb[:], zslot(tt, 3))]

        def finish2A(tt, ps):
            drain(tt, 1, ps)

        def finish2B(tt, ps):
            drain(tt, 1, ps)
            if tt == nt - WGRP // 2 - 1:
                write_half(2, 1, tt // WGRP, WGRP // 2)
            elif tt == nt - 1:
                g0 = nt - WGRP // 2
                w = nc.scalar.dma_start(
                    bsh[2][:, bass.ds(off_by_eng["act"] + g0, WGRP // 2), :],
                    z_sb[:, g0:g0 + WGRP // 2, 1, :, :]
                    .rearrange("p t b o -> p t (b o)"))
                shared_writes[2].append(w)
            elif (tt + 1) % WGRP == 0:
                write_half(2, 1, tt // WGRP)

        spmm_phase(bsh[1], gval2_sb, cc1, seeds2A, finish2A, seeds2B, finish2B)

        # phase 3: out = (z0 + L_own b1) + L_peer b1 - b2   (bias already in z0)
        cc2 = pair_barrier(2)

        def seeds3A(tt):
            return [(ident_sb[:], zslot(tt, 0))]

        def seeds3B(tt):
            return [(ident_sb[:], zslot(tt, 0)),
                    (nident_sb[:], zslot(tt, 2))]

        def finish3A(tt, ps):
            drain(tt, 0, ps)

        ostate = {"ot": None}

        def finish3B(tt, ps):
            if tt % WGRP == 0:
                ot_new = opool.tile([P, WGRP, F], BF16, tag="ot")
                ostate["ot"] = ot_new
            ot = ostate["ot"]
            nc.scalar.activation(out=ot[:, tt % WGRP, :], in_=ps[:],
                                 func=mybir.ActivationFunctionType.Copy)
            if (tt + 1) % WGRP == 0:
                g0 = (tt // WGRP) * WGRP
                nc.sync.dma_start(out_d[:, g0:g0 + WGRP, :], ot[:])

        spmm_phase(bsh[2], gval1_sb, cc2, seeds3A, finish3A, seeds3B, finish3B)

    nc.compile()
    return nc


def make_host_inputs(inputs, weight, bias, lap_vals, lap_rows, lap_cols):
    per_parity, chunks = _preprocess_lap(
        np.asarray(lap_rows), np.asarray(lap_cols),
        np.asarray(lap_vals, np.float32))
    w = np.asarray(weight, np.float32)
    # wz[(t,f) split cc, (k,o)]
    wz = np.transpose(w, (2, 0, 1, 3)).reshape(C, KV * FOUT)
    wz = np.ascontiguousarray(
        wz.reshape(2, P, KV * FOUT).transpose(1, 0, 2)).astype(ml_dtypes.bfloat16)
    biasw = np.zeros((1, KV * FOUT), np.float32)
    biasw[0, :FOUT] = np.asarray(bias, np.float32)
    biasw = biasw.astype(ml_dtypes.bfloat16)
    onesb = np.ones((1, P), ml_dtypes.bfloat16)
    ident128 = np.eye(P, dtype=np.float32).astype(ml_dtypes.bfloat16)
    iota128 = np.ascontiguousarray(
        np.broadcast_to(np.arange(P, dtype=np.float32)[None, :],
                        (P, P))).astype(ml_dtypes.bfloat16)
    x = np.asarray(inputs, np.float32)
    in_maps = []
    for r in range(N_CORES):
        pair, h = r // 2, r % 2
        gidx_w, growl_m, gval_m = per_parity[h]
        # xt[b, cc, cl, v] = x[4p+b, h*VH + v, t, f], c=(t,f)=cc*128+cl
        xs = x[BG * pair:BG * (pair + 1), h * VH:(h + 1) * VH]  # [4, VH, T, FIN]
        xt = xs.reshape(BG, VH, C).transpose(0, 2, 1).reshape(BG, 2, P, VH)
        m = {
            "xt": np.ascontiguousarray(xt).astype(ml_dtypes.bfloat16),
            "wz": wz,
            "biasw": biasw,
            "onesb": onesb,
            "iota128": iota128,
            "ident128": ident128,
            "nident128": -ident128,
            "offt": np.array([[h * NT]], np.int32),
            "gidx": gidx_w,
            "growl": growl_m,
            "gval1": gval_m,
            "gval2": np.ascontiguousarray(2.0 * gval_m),
        }
        in_maps.append(m)
    return in_maps, chunks


_CACHE = {}


def _get_program(chunks, has_bias):
    key = (tuple(chunks[0]), tuple(chunks[1]), has_bias)
    if key not in _CACHE:
        _CACHE[key] = build_program((list(chunks[0]), list(chunks[1])), has_bias)
    return _CACHE[key]


def kernel(inputs, weight, bias, lap_vals, lap_rows, lap_cols):
    in_maps, chunks = make_host_inputs(inputs, weight, bias, lap_vals,
                                       lap_rows, lap_cols)
    nc = _get_program(chunks, bool(np.any(np.asarray(bias))))
    res = run_bass_kernel_spmd(nc, in_maps, list(range(N_CORES)))
    out = np.empty((B, V, FOUT), np.float32)
    for r in range(N_CORES):
        pair, h = r // 2, r % 2
        o = np.asarray(res.results[r]["out"], np.float32).reshape(P, NT, BG, FOUT)
        o = o.transpose(2, 1, 0, 3).reshape(BG, VH, FOUT)
        out[BG * pair:BG * (pair + 1), h * VH:(h + 1) * VH, :] = o
    return np.ascontiguousarray(out)


def time_kernel(inputs_dict, iters=3):
    """Wall-clock repeated executions of the cached program (ns per run)."""
    import time

    in_maps, chunks = make_host_inputs(**inputs_dict)
    nc = _get_program(chunks, bool(np.any(np.asarray(inputs_dict["bias"]))))
    times = []
    for _ in range(iters):
        t0 = time.perf_counter()
        run_bass_kernel_spmd(nc, in_maps, list(range(N_CORES)))
        times.append(time.perf_counter() - t0)
    return min(times) * 1e9


# revision 52
# speedup vs baseline: 1.0324x; 1.0133x over previous
"""Trainium2 Bass kernel for ConvChebTemp (Chebyshev graph conv, temporal weights).

Math: out[b,v,o] = sum_{k,t,f} T_k(L)x0[:,t,f,b] w[f,k,t,o] + bias[o]
with x0 = inputs permuted to [V, T*Fin*B] and T_k the Chebyshev recurrence.

Clenshaw reformulation (weights contracted first):
  z_k[v,b,o] = sum_{t,f} x0[v,t,f,b] w[f,k,t,o]
  b3 = z3; b2 = z2 + 2 L b3; b1 = z1 + 2 L b2 - b3; out = z0 + L b1 - b2 + bias

Sharding: 8 cores = 4 pairs. Pair p owns batches [4p, 4p+4); within the pair
the graph rows are split in half (core 2p: rows [0, V/2), core 2p+1 the rest).
The Clenshaw iterates b3/b2/b1 live in pair-SHARED HBM tensors
(addr_space="Shared": cores (2k, 2k+1) see one physical buffer), so each
core writes only its half and gathers from the full tensor. Cross-core
ordering is a tiny per-pair AllGather barrier before each phase's gathers.

Everything on the SpMM path is bf16: gather rows are 4 batches x 64 Fout x 2B
= 512B (full DMA descriptor efficiency) and all matmuls run at 1 cycle/row.
"""
import sys

sys.path.insert(0, "/opt/trn_rl_repo")

from contextlib import ExitStack  # noqa: E402

import ml_dtypes  # noqa: E402
import numpy as np  # noqa: E402

from concourse import bacc, bass, mybir, tile  # noqa: E402
from concourse.bass_utils import run_bass_kernel_spmd  # noqa: E402

P = 128
N_CORES = 8
FP32 = mybir.dt.float32
BF16 = mybir.dt.bfloat16
I32 = mybir.dt.int32
I16 = mybir.dt.int16

# Problem dims (hardcoded per spec)
B, V, T, FIN = 16, 12288, 4, 64
KV, KT, FOUT = 4, 4, 64
VH = V // 2                # rows per core
NT = VH // P               # out-tiles per core (48)
BG = 4                     # batches per pair
F = BG * FOUT              # spmm row width (256 bf16 = 512B)
C = T * FIN                # z contraction dim (256)
PAIR_GROUPS = [[0, 1], [2, 3], [4, 5], [6, 7]]
CHUNKS_PER_PIECE = 8       # 1024 gather indices per instruction
DMA_SCRATCH = 16384        # SWDGE ring: 1024 descriptors
WGRP = 8                   # out-tiles per batched shared-HBM write


def _preprocess_lap(lap_rows, lap_cols, lap_vals):
    """Split nnz by row-half into own-column (section A) and peer-column
    (section B) chunk streams, padded to a common per-tile chunk structure
    (identical across cores so one SPMD program serves all).

    Section A only references rows this core wrote itself, so its gathers
    need no cross-core barrier.

    Returns (per_parity list of (gidx_wrapped, growl, gval),
    (chunksA_per_tile, chunksB_per_tile)).
    """
    halves = []
    cnt = np.zeros((2, 2, NT), np.int64)  # [section, parity, tile]
    for h in (0, 1):
        lo, hi = h * VH, (h + 1) * VH
        m = (lap_rows >= lo) & (lap_rows < hi)
        lrows = lap_rows[m] - lo
        order = np.argsort(lrows, kind="stable")
        lrows = lrows[order]
        cols = lap_cols[m][order]
        vals = lap_vals[m][order]
        own = (cols >= lo) & (cols < hi)
        tiles = lrows // P
        np.add.at(cnt[0, h], tiles[own], 1)
        np.add.at(cnt[1, h], tiles[~own], 1)
        halves.append((lrows, cols, vals, own, tiles))
    # pass A gets only FULL chunks of own-column nnz (min across parities so
    # neither pads); leftovers ride in pass B's first chunk, which is gathered
    # after the barrier anyway. This keeps total chunks near the unsplit count.
    chunksA = [min(int(cnt[0, 0][t] // P), int(cnt[0, 1][t] // P))
               for t in range(NT)]
    chunksB = [max(1,
                   int(-(-(cnt[0, 0][t] - P * chunksA[t] + cnt[1, 0][t]) // P)),
                   int(-(-(cnt[0, 1][t] - P * chunksA[t] + cnt[1, 1][t]) // P)))
               for t in range(NT)]
    nchunk = sum(chunksA) + sum(chunksB)
    nnzp = nchunk * P
    out = []
    for h, (lrows, cols, vals, own, tiles) in enumerate(halves):
        # pad slots must gather an own-half row (peer half may be unwritten
        # while section A streams): local row 0 of my half
        pad_v = h * VH
        gidx = np.full(nnzp, pad_v, np.int32)
        growl = np.zeros(nnzp, np.float32)
        gval = np.zeros(nnzp, np.float32)
        pos = 0
        for t in range(NT):  # section A: first P*chunksA[t] own nnz
            m = own & (tiles == t)
            n = P * chunksA[t]
            idx = np.flatnonzero(m)[:n]
            assert len(idx) == n
            gidx[pos:pos + n] = cols[idx]
            growl[pos:pos + n] = (lrows[idx] - t * P).astype(np.float32)
            gval[pos:pos + n] = vals[idx]
            pos += n
        for t in range(NT):  # section B: leftover own + all peer nnz
            m = own & (tiles == t)
            skip = P * chunksA[t]
            idx = np.concatenate([np.flatnonzero(m)[skip:],
                                  np.flatnonzero((~own) & (tiles == t))])
            n = len(idx)
            gidx[pos:pos + n] = cols[idx]
            growl[pos:pos + n] = (lrows[idx] - t * P).astype(np.float32)
            gval[pos:pos + n] = vals[idx]
            pos += chunksB[t] * P
        assert pos == nnzp
        # remap to partition-major rows: v -> (v % 128) * 96 + v // 128
        gidx = ((gidx % P) * (V // P) + gidx // P).astype(np.int16)
        gidx_w = np.tile(gidx.reshape(-1, 16).T.copy(), (8, 1))  # [128, nnzp/16]
        growl_m = growl.reshape(nchunk, P).T.copy()
        gval_m = gval.reshape(nchunk, P).T.copy()
        out.append((np.ascontiguousarray(gidx_w),
                    np.ascontiguousarray(growl_m),
                    np.ascontiguousarray(gval_m)))
    return out, (chunksA, chunksB)


def build_program(chunks_per_tile, has_bias, n_cores=N_CORES):
    nt = NT
    nchunk = sum(chunks_per_tile[0]) + sum(chunks_per_tile[1])
    nnzp = nchunk * P
    nc = bacc.Bacc("TRN2", target_bir_lowering=False, debug=False,
                   num_devices=n_cores, dynamic_dma_scratch_size=DMA_SCRATCH)

    xt_d = nc.dram_tensor("xt", [BG, 2, P, VH], BF16, kind="ExternalInput")
    wz_d = nc.dram_tensor("wz", [P, 2, KV * FOUT], BF16, kind="ExternalInput")
    onesb_d = nc.dram_tensor("onesb", [1, P], BF16, kind="ExternalInput")
    biasw_d = nc.dram_tensor("biasw", [1, KV * FOUT], BF16, kind="ExternalInput")
    iota_d = nc.dram_tensor("iota128", [P, P], BF16, kind="ExternalInput")
    ident_d = nc.dram_tensor("ident128", [P, P], BF16, kind="ExternalInput")
    nident_d = nc.dram_tensor("nident128", [P, P], BF16, kind="ExternalInput")
    offt_d = nc.dram_tensor("offt", [1, 1], I32, kind="ExternalInput")
    gidx_d = nc.dram_tensor("gidx", [P, nnzp // 16], I16, kind="ExternalInput")
    growl_d = nc.dram_tensor("growl", [P, nchunk], FP32, kind="ExternalInput")
    gval1_d = nc.dram_tensor("gval1", [P, nchunk], FP32, kind="ExternalInput")
    gval2_d = nc.dram_tensor("gval2", [P, nchunk], FP32, kind="ExternalInput")
    out_d = nc.dram_tensor("out", [P, NT, F], BF16, kind="ExternalOutput")

    # pair-shared Clenshaw iterates (both cores of a pair see one buffer),
    # stored partition-major: row v lives at [v % 128, v // 128, :] so the
    # per-core half writes are 128 contiguous 4KB descriptors per group
    bsh = [nc.dram_tensor(f"bsh{k}", [P, V // P, F], BF16, kind="Internal",
                          addr_space="Shared") for k in range(3)]
    bin_d = [nc.dram_tensor(f"bin{k}", [1, 16], BF16, kind="Internal")
             for k in range(3)]
    bout_d = [nc.dram_tensor(f"bout{k}", [2, 16], BF16, kind="Internal")
              for k in range(3)]

    with tile.TileContext(nc) as tc, ExitStack() as ctx:
        const = ctx.enter_context(tc.tile_pool(name="const", bufs=1))
        zres = ctx.enter_context(tc.tile_pool(name="zres", bufs=1))
        xpool = ctx.enter_context(tc.tile_pool(name="x", bufs=2))
        gpool = ctx.enter_context(tc.tile_pool(name="gbuf", bufs=8))
        spool = ctx.enter_context(tc.tile_pool(name="sel", bufs=4))
        opool = ctx.enter_context(tc.tile_pool(name="ostg", bufs=2))
        bpool = ctx.enter_context(tc.tile_pool(name="bounce", bufs=1))
        psz = ctx.enter_context(tc.tile_pool(name="psz", bufs=4, space="PSUM"))
        pss = ctx.enter_context(tc.tile_pool(name="pss", bufs=4, space="PSUM"))

        # constants + metadata resident in SBUF
        iota_sb = const.tile([P, P], BF16, tag="iota")
        nc.sync.dma_start(iota_sb[:], iota_d[:, :])
        ident_sb = const.tile([P, P], BF16, tag="ident")
        nc.sync.dma_start(ident_sb[:], ident_d[:, :])
        nident_sb = const.tile([P, P], BF16, tag="nident")
        nc.sync.dma_start(nident_sb[:], nident_d[:, :])
        ones_sb = const.tile([1, P], BF16, tag="ones")
        nc.sync.dma_start(ones_sb[:], onesb_d[:, :])
        biasw_sb = const.tile([1, KV * FOUT], BF16, tag="biasw")
        nc.sync.dma_start(biasw_sb[:], biasw_d[:, :])
        wz_sb = const.tile([P, 2, KV * FOUT], BF16, tag="wz")
        nc.sync.dma_start(wz_sb[:], wz_d[:, :, :])
        gidx_sb = const.tile([P, nnzp // 16], I16, tag="gidx")
        nc.sync.dma_start(gidx_sb[:], gidx_d[:, :])
        growl_sb = const.tile([P, nchunk], FP32, tag="growl")
        nc.sync.dma_start(growl_sb[:], growl_d[:, :])
        gval1_sb = const.tile([P, nchunk], FP32, tag="gval1")
        nc.sync.dma_start(gval1_sb[:], gval1_d[:, :])
        gval2_sb = const.tile([P, nchunk], FP32, tag="gval2")
        nc.sync.dma_start(gval2_sb[:], gval2_d[:, :])

        # my tile offset into the shared tensors (0 or NT); loaded on both
        # engines that issue symbolic shared writes (SP for the Z-phase b3
        # writes, Activation for the spmm-phase writes)
        off_by_eng = {}
        for eng, nm in ((nc.scalar, "act"), (nc.sync, "sp")):
            off_reg = eng.alloc_register(f"slab_off_{nm}")
            eng.reg_load(off_reg, offt_d[0:1, 0:1])
            off_by_eng[nm] = eng.snap(off_reg, donate=True, min_val=0,
                                      max_val=NT)

        # all z_k resident in SBUF: [P, nt, KV, BG, FOUT] bf16 (96KB/partition)
        z_sb = zres.tile([P, nt, KV, BG, FOUT], BF16, tag="z")

        shared_writes = {0: [], 1: [], 2: []}

        def write_half(kidx, kslot, grp, ntiles=WGRP):
            """Batched write of ntiles tiles of z-slot kslot to shared bsh[kidx]."""
            g0 = grp * WGRP
            eng, off = ((nc.sync, "sp") if kidx == 0 else (nc.scalar, "act"))
            dst = bsh[kidx][:, bass.ds(off_by_eng[off] + g0, ntiles), :]
            src = z_sb[:, g0:g0 + ntiles, kslot, :, :] \
                .rearrange("p t b o -> p t (b o)")
            w = eng.dma_start(dst, src)
            shared_writes[kidx].append(w)

        # ---------- phase Z: z_k = x0 @ w_k (+ bias folded into z0) ----------
        VHH = VH // 2
        for b in range(BG):
          for half in range(2):
            v0 = half * VHH
            xb = xpool.tile([P, 2, VHH], BF16, tag="xb")
            nc.sync.dma_start(
                xb[:], xt_d[b, :, :, v0:v0 + VHH].rearrange("c p v -> p c v"))
            for vt0 in range(half * nt // 2, (half + 1) * nt // 2, 2):
                zps = psz.tile([P, 2, KV * FOUT], FP32, tag="zps")
                for sub in range(2):
                    vt = vt0 + sub
                    for cc in range(2):
                        nc.tensor.matmul(
                            zps[:, sub, :],
                            lhsT=xb[:, cc, vt * P - v0:(vt + 1) * P - v0],
                            rhs=wz_sb[:, cc, :],
                            start=(cc == 0),
                            stop=(cc == 1 and not has_bias))
                    if has_bias:
                        nc.tensor.matmul(zps[:, sub, :], lhsT=ones_sb[:, :],
                                         rhs=biasw_sb[:, :], start=False,
                                         stop=True)
                # PSUM->SBUF cast copies: DVE 1/3, Act 2/3 (Act is cheaper)
                if (vt0 // 2) % 3 == 0:
                    nc.vector.tensor_copy(
                        z_sb[:, vt0:vt0 + 2, :, b, :],
                        zps[:].rearrange("p s (k o) -> p s k o", o=FOUT))
                else:
                    nc.scalar.activation(
                        out=z_sb[:, vt0:vt0 + 2, :, b, :],
                        in_=zps[:].rearrange("p s (k o) -> p s k o", o=FOUT),
                        func=mybir.ActivationFunctionType.Copy)
                if b == BG - 1 and (vt0 + 2) % WGRP == 0:
                    write_half(0, 3, vt0 // WGRP)

        def pair_barrier(k):
            # the AllGather is a pure rendezvous: gate it on ALL my shared
            # writes; completion proves the peer's writes are done too (the
            # payload itself is never read)
            cc = nc.gpsimd.collective_compute(
                "AllGather", mybir.AluOpType.bypass, PAIR_GROUPS,
                ins=[bin_d[k][0:1, :]], outs=[bout_d[k][:, :]])
            for w in shared_writes[k]:
                bass._add_dep_helper(cc.ins, w.ins, sync=True,
                                     reason="barrier after all shared writes")
            return cc

        # ---------- spmm phases ----------
        # Each phase runs in two passes: pass A covers own-half columns
        # (rows this core wrote -> no cross-core barrier; overlaps with the
        # AllGather rendezvous), pass B covers peer-half columns and waits
        # on the barrier. z-slots accumulate partials between the passes.
        chunksA, chunksB = chunks_per_tile
        nA = sum(chunksA)

        def spmm_pass(src_d, vals_sb, base0, chunks_list, nend, dep_inst,
                      seeds, finish):
            state = {"gb": None, "base": base0, "len": 0}

            def ensure_piece(c):
                while state["gb"] is None or c >= state["base"] + state["len"]:
                    base = (base0 if state["gb"] is None
                            else state["base"] + state["len"])
                    plen = min(CHUNKS_PER_PIECE, nend - base)
                    gb = gpool.tile([P, plen, F], BF16, tag="gb")
                    s0 = base * P
                    nidx = plen * P
                    g = nc.gpsimd.dma_gather(
                        out_ap=gb[:],
                        in_ap=src_d[:, :, :].rearrange("p t f -> (p t) f"),
                        idxs_ap=gidx_sb[:, s0 // 16:(s0 + nidx) // 16],
                        num_idxs=nidx,
                        num_idxs_reg=nidx,
                        elem_size=F,
                    )
                    if dep_inst is not None:
                        bass._add_dep_helper(g.ins, dep_inst.ins, sync=True,
                                             reason="pair barrier before gather")
                    state.update(gb=gb, base=base, len=plen)
                return state["gb"], state["base"]

            ci = base0
            for tt in range(nt):
                nck = chunks_list[tt]
                if nck == 0:
                    continue
                ps = pss.tile([P, F], FP32, tag="ps")
                # seed the accumulator with the running z-slot value(s) so
                # the DVE never has to do the adds
                sds = seeds(tt)
                for si, (w, src) in enumerate(sds):
                    nc.tensor.matmul(ps[:], lhsT=w, rhs=src,
                                     start=(si == 0), stop=False)
                for k in range(nck):
                    col = ci + k
                    gb, base = ensure_piece(col)
                    sT = spool.tile([P, P], BF16, tag="sT")
                    nc.vector.tensor_scalar(
                        out=sT[:], in0=iota_sb[:],
                        scalar1=growl_sb[:, col:col + 1],
                        scalar2=vals_sb[:, col:col + 1],
                        op0=mybir.AluOpType.is_equal,
                        op1=mybir.AluOpType.mult,
                    )
                    nc.tensor.matmul(ps[:], lhsT=sT[:], rhs=gb[:, col - base, :],
                                     start=False, stop=(k == nck - 1))
                finish(tt, ps)
                ci += nck

        def spmm_phase(src_d, vals_sb, cc_inst, seedsA, finishA, seedsB,
                       finishB):
            spmm_pass(src_d, vals_sb, 0, chunksA, nA, None, seedsA, finishA)
            spmm_pass(src_d, vals_sb, nA, chunksB, nchunk, cc_inst, seedsB,
                      finishB)

        def zslot(vt, k):
            return z_sb[:, vt, k, :, :].rearrange("p b o -> p (b o)")

        def ps3(ps):
            return ps[:].rearrange("p (b o) -> p b o", o=FOUT)

        def drain(tt, k, ps):
            nc.scalar.activation(out=zslot(tt, k), in_=ps[:],
                                 func=mybir.ActivationFunctionType.Copy)

        # phase 1: b2 = z2 + 2 L b3   (result overwrites z2 slot)
        cc0 = pair_barrier(0)

        def seeds1(tt):
            return [(ident_sb[:], zslot(tt, 2))]

        def finish1A(tt, ps):
            drain(tt, 2, ps)

        def finish1B(tt, ps):
            drain(tt, 2, ps)
            if tt == nt - WGRP // 2 - 1:
                write_half(1, 2, tt // WGRP, WGRP // 2)
            elif tt == nt - 1:
                g0 = nt - WGRP // 2
                w = nc.scalar.dma_start(
                    bsh[1][:, bass.ds(off_by_eng["act"] + g0, WGRP // 2), :],
                    z_sb[:, g0:g0 + WGRP // 2, 2, :, :]
                    .rearrange("p t b o -> p t (b o)"))
                shared_writes[1].append(w)
            elif (tt + 1) % WGRP == 0:
                write_half(1, 2, tt // WGRP)

        spmm_phase(bsh[0], gval2_sb, cc0, seeds1, finish1A, seeds1, finish1B)

        # phase 2: b1 = z1 + 2 L b2 - b3   (result overwrites z1 slot)
        cc1 = pair_barrier(1)

        def seeds2A(tt):
            return [(ident_sb[:], zslot(tt, 1))]

        def seeds2B(tt):
            return [(ident_sb[:], zslot(tt, 1)),
                    (nident_sb[:], zslot(tt, 3))]

        def finish2A(tt, ps):
            drain(tt, 1, ps)

        def finish2B(tt, ps):
            drain(tt, 1, ps)
            if tt == nt - WGRP // 2 - 1:
                write_half(2, 1, tt // WGRP, WGRP // 2)
            elif tt == nt - 1:
                g0 = nt - WGRP // 2
                w = nc.scalar.dma_start(
                    bsh[2][:, bass.ds(off_by_eng["act"] + g0, WGRP // 2), :],
                    z_sb[:, g0:g0 + WGRP // 2, 1, :, :]
                    .rearrange("p t b o -> p t (b o)"))
                shared_writes[2].append(w)
            elif (tt + 1) % WGRP == 0:
                write_half(2, 1, tt // WGRP)

        spmm_phase(bsh[1], gval2_sb, cc1, seeds2A, finish2A, seeds2B, finish2B)

        # phase 3: out = (z0 + L_own b1) + L_peer b1 - b2   (bias already in z0)
        cc2 = pair_barrier(2)

        def seeds3A(tt):
            return [(ident_sb[:], zslot(tt, 0))]

        def seeds3B(tt):
            return [(ident_sb[:], zslot(tt, 0)),
                    (nident_sb[:], zslot(tt, 2))]

        def finish3A(tt, ps):
            drain(tt, 0, ps)

        ostate = {"ot": None}

        def finish3B(tt, ps):
            if tt % WGRP == 0:
                ot_new = opool.tile([P, WGRP, F], BF16, tag="ot")
                ostate["ot"] = ot_new
            ot = ostate["ot"]
            nc.scalar.activation(out=ot[:, tt % WGRP, :], in_=ps[:],
                                 func=mybir.ActivationFunctionType.Copy)
            if (tt + 1) % WGRP == 0:
                g0 = (tt // WGRP) * WGRP
                nc.sync.dma_start(out_d[:, g0:g0 + WGRP, :], ot[:])

        spmm_phase(bsh[2], gval1_sb, cc2, seeds3A, finish3A, seeds3B, finish3B)

    nc.compile()
    return nc


def make_host_inputs(inputs, weight, bias, lap_vals, lap_rows, lap_cols):
    per_parity, chunks = _preprocess_lap(
        np.asarray(lap_rows), np.asarray(lap_cols),
        np.asarray(lap_vals, np.float32))
    w = np.asarray(weight, np.float32)
    # wz[(t,f) split cc, (k,o)]
    wz = np.transpose(w, (2, 0, 1, 3)).reshape(C, KV * FOUT)
    wz = np.ascontiguousarray(
        wz.reshape(2, P, KV * FOUT).transpose(1, 0, 2)).astype(ml_dtypes.bfloat16)
    biasw = np.zeros((1, KV * FOUT), np.float32)
    biasw[0, :FOUT] = np.asarray(bias, np.float32)
    biasw = biasw.astype(ml_dtypes.bfloat16)
    onesb = np.ones((1, P), ml_dtypes.bfloat16)
    ident128 = np.eye(P, dtype=np.float32).astype(ml_dtypes.bfloat16)
    iota128 = np.ascontiguousarray(
        np.broadcast_to(np.arange(P, dtype=np.float32)[None, :],
                        (P, P))).astype(ml_dtypes.bfloat16)
    x = np.asarray(inputs, np.float32)
    in_maps = []
    for r in range(N_CORES):
        pair, h = r // 2, r % 2
        gidx_w, growl_m, gval_m = per_parity[h]
        # xt[b, cc, cl, v] = x[4p+b, h*VH + v, t, f], c=(t,f)=cc*128+cl
        xs = x[BG * pair:BG * (pair + 1), h * VH:(h + 1) * VH]  # [4, VH, T, FIN]
        xt = xs.reshape(BG, VH, C).transpose(0, 2, 1).reshape(BG, 2, P, VH)
        m = {
            "xt": np.ascontiguousarray(xt).astype(ml_dtypes.bfloat16),
            "wz": wz,
            "biasw": biasw,
            "onesb": onesb,
            "iota128": iota128,
            "ident128": ident128,
            "nident128": -ident128,
            "offt": np.array([[h * NT]], np.int32),
            "gidx": gidx_w,
            "growl": growl_m,
            "gval1": gval_m,
            "gval2": np.ascontiguousarray(2.0 * gval_m),
        }
        in_maps.append(m)
    return in_maps, chunks


_CACHE = {}


def _get_program(chunks, has_bias):
    key = (tuple(chunks[0]), tuple(chunks[1]), has_bias)
    if key not in _CACHE:
        _CACHE[key] = build_program((list(chunks[0]), list(chunks[1])), has_bias)
    return _CACHE[key]


def kernel(inputs, weight, bias, lap_vals, lap_rows, lap_cols):
    in_maps, chunks = make_host_inputs(inputs, weight, bias, lap_vals,
                                       lap_rows, lap_cols)
    nc = _get_program(chunks, bool(np.any(np.asarray(bias))))
    res = run_bass_kernel_spmd(nc, in_maps, list(range(N_CORES)))
    out = np.empty((B, V, FOUT), np.float32)
    for r in range(N_CORES):
        pair, h = r // 2, r % 2
        o = np.asarray(res.results[r]["out"], np.float32).reshape(P, NT, BG, FOUT)
        o = o.transpose(2, 1, 0, 3).reshape(BG, VH, FOUT)
        out[BG * pair:BG * (pair + 1), h * VH:(h + 1) * VH, :] = o
    return np.ascontiguousarray(out)


def time_kernel(inputs_dict, iters=3):
    """Wall-clock repeated executions of the cached program (ns per run)."""
    import time

    in_maps, chunks = make_host_inputs(**inputs_dict)
    nc = _get_program(chunks, bool(np.any(np.asarray(inputs_dict["bias"]))))
    times = []
    for _ in range(iters):
        t0 = time.perf_counter()
        run_bass_kernel_spmd(nc, in_maps, list(range(N_CORES)))
        times.append(time.perf_counter() - t0)
    return min(times) * 1e9


# revision 53
# speedup vs baseline: 1.0371x; 1.0046x over previous
"""Trainium2 Bass kernel for ConvChebTemp (Chebyshev graph conv, temporal weights).

Math: out[b,v,o] = sum_{k,t,f} T_k(L)x0[:,t,f,b] w[f,k,t,o] + bias[o]
with x0 = inputs permuted to [V, T*Fin*B] and T_k the Chebyshev recurrence.

Clenshaw reformulation (weights contracted first):
  z_k[v,b,o] = sum_{t,f} x0[v,t,f,b] w[f,k,t,o]
  b3 = z3; b2 = z2 + 2 L b3; b1 = z1 + 2 L b2 - b3; out = z0 + L b1 - b2 + bias

Sharding: 8 cores = 4 pairs. Pair p owns batches [4p, 4p+4); within the pair
the graph rows are split in half (core 2p: rows [0, V/2), core 2p+1 the rest).
The Clenshaw iterates b3/b2/b1 live in pair-SHARED HBM tensors
(addr_space="Shared": cores (2k, 2k+1) see one physical buffer), so each
core writes only its half and gathers from the full tensor. Cross-core
ordering is a tiny per-pair AllGather barrier before each phase's gathers.

Everything on the SpMM path is bf16: gather rows are 4 batches x 64 Fout x 2B
= 512B (full DMA descriptor efficiency) and all matmuls run at 1 cycle/row.
"""
import sys

sys.path.insert(0, "/opt/trn_rl_repo")

from contextlib import ExitStack  # noqa: E402

import ml_dtypes  # noqa: E402
import numpy as np  # noqa: E402

from concourse import bacc, bass, mybir, tile  # noqa: E402
from concourse.bass_utils import run_bass_kernel_spmd  # noqa: E402

P = 128
N_CORES = 8
FP32 = mybir.dt.float32
BF16 = mybir.dt.bfloat16
I32 = mybir.dt.int32
I16 = mybir.dt.int16

# Problem dims (hardcoded per spec)
B, V, T, FIN = 16, 12288, 4, 64
KV, KT, FOUT = 4, 4, 64
VH = V // 2                # rows per core
NT = VH // P               # out-tiles per core (48)
BG = 4                     # batches per pair
F = BG * FOUT              # spmm row width (256 bf16 = 512B)
C = T * FIN                # z contraction dim (256)
PAIR_GROUPS = [[0, 1], [2, 3], [4, 5], [6, 7]]
CHUNKS_PER_PIECE = 8       # 1024 gather indices per instruction
DMA_SCRATCH = 16384        # SWDGE ring: 1024 descriptors
WGRP = 8                   # out-tiles per batched shared-HBM write


def _preprocess_lap(lap_rows, lap_cols, lap_vals):
    """Split nnz by row-half into own-column (section A) and peer-column
    (section B) chunk streams, padded to a common per-tile chunk structure
    (identical across cores so one SPMD program serves all).

    Section A only references rows this core wrote itself, so its gathers
    need no cross-core barrier.

    Returns (per_parity list of (gidx_wrapped, growl, gval),
    (chunksA_per_tile, chunksB_per_tile)).
    """
    halves = []
    cnt = np.zeros((2, 2, NT), np.int64)  # [section, parity, tile]
    for h in (0, 1):
        lo, hi = h * VH, (h + 1) * VH
        m = (lap_rows >= lo) & (lap_rows < hi)
        lrows = lap_rows[m] - lo
        order = np.argsort(lrows, kind="stable")
        lrows = lrows[order]
        cols = lap_cols[m][order]
        vals = lap_vals[m][order]
        own = (cols >= lo) & (cols < hi)
        tiles = lrows // P
        np.add.at(cnt[0, h], tiles[own], 1)
        np.add.at(cnt[1, h], tiles[~own], 1)
        halves.append((lrows, cols, vals, own, tiles))
    # pass A gets only FULL chunks of own-column nnz (min across parities so
    # neither pads); leftovers ride in pass B's first chunk, which is gathered
    # after the barrier anyway. This keeps total chunks near the unsplit count.
    chunksA = [min(int(cnt[0, 0][t] // P), int(cnt[0, 1][t] // P))
               for t in range(NT)]
    chunksB = [max(1,
                   int(-(-(cnt[0, 0][t] - P * chunksA[t] + cnt[1, 0][t]) // P)),
                   int(-(-(cnt[0, 1][t] - P * chunksA[t] + cnt[1, 1][t]) // P)))
               for t in range(NT)]
    nchunk = sum(chunksA) + sum(chunksB)
    nnzp = nchunk * P
    out = []
    for h, (lrows, cols, vals, own, tiles) in enumerate(halves):
        # pad slots must gather an own-half row (peer half may be unwritten
        # while section A streams): local row 0 of my half
        pad_v = h * VH
        gidx = np.full(nnzp, pad_v, np.int32)
        growl = np.zeros(nnzp, np.float32)
        gval = np.zeros(nnzp, np.float32)
        pos = 0
        for t in range(NT):  # section A: first P*chunksA[t] own nnz
            m = own & (tiles == t)
            n = P * chunksA[t]
            idx = np.flatnonzero(m)[:n]
            assert len(idx) == n
            gidx[pos:pos + n] = cols[idx]
            growl[pos:pos + n] = (lrows[idx] - t * P).astype(np.float32)
            gval[pos:pos + n] = vals[idx]
            pos += n
        for t in range(NT):  # section B: leftover own + all peer nnz
            m = own & (tiles == t)
            skip = P * chunksA[t]
            idx = np.concatenate([np.flatnonzero(m)[skip:],
                                  np.flatnonzero((~own) & (tiles == t))])
            n = len(idx)
            gidx[pos:pos + n] = cols[idx]
            growl[pos:pos + n] = (lrows[idx] - t * P).astype(np.float32)
            gval[pos:pos + n] = vals[idx]
            pos += chunksB[t] * P
        assert pos == nnzp
        # remap to partition-major rows: v -> (v % 128) * 96 + v // 128
        gidx = ((gidx % P) * (V // P) + gidx // P).astype(np.int16)
        gidx_w = np.tile(gidx.reshape(-1, 16).T.copy(), (8, 1))  # [128, nnzp/16]
        growl_m = growl.reshape(nchunk, P).T.copy()
        gval_m = gval.reshape(nchunk, P).T.copy()
        out.append((np.ascontiguousarray(gidx_w),
                    np.ascontiguousarray(growl_m),
                    np.ascontiguousarray(gval_m)))
    return out, (chunksA, chunksB)


def build_program(chunks_per_tile, has_bias, n_cores=N_CORES):
    nt = NT
    nchunk = sum(chunks_per_tile[0]) + sum(chunks_per_tile[1])
    nnzp = nchunk * P
    nc = bacc.Bacc("TRN2", target_bir_lowering=False, debug=False,
                   num_devices=n_cores, dynamic_dma_scratch_size=DMA_SCRATCH)

    xt_d = nc.dram_tensor("xt", [BG, 2, P, VH], BF16, kind="ExternalInput")
    wz_d = nc.dram_tensor("wz", [P, 2, KV * FOUT], BF16, kind="ExternalInput")
    onesb_d = nc.dram_tensor("onesb", [1, P], BF16, kind="ExternalInput")
    biasw_d = nc.dram_tensor("biasw", [1, KV * FOUT], BF16, kind="ExternalInput")
    iota_d = nc.dram_tensor("iota128", [P, P], BF16, kind="ExternalInput")
    ident_d = nc.dram_tensor("ident128", [P, P], BF16, kind="ExternalInput")
    nident_d = nc.dram_tensor("nident128", [P, P], BF16, kind="ExternalInput")
    offt_d = nc.dram_tensor("offt", [1, 1], I32, kind="ExternalInput")
    gidx_d = nc.dram_tensor("gidx", [P, nnzp // 16], I16, kind="ExternalInput")
    growl_d = nc.dram_tensor("growl", [P, nchunk], FP32, kind="ExternalInput")
    gval1_d = nc.dram_tensor("gval1", [P, nchunk], FP32, kind="ExternalInput")
    gval2_d = nc.dram_tensor("gval2", [P, nchunk], FP32, kind="ExternalInput")
    out_d = nc.dram_tensor("out", [P, NT, F], BF16, kind="ExternalOutput")

    # pair-shared Clenshaw iterates (both cores of a pair see one buffer),
    # stored partition-major: row v lives at [v % 128, v // 128, :] so the
    # per-core half writes are 128 contiguous 4KB descriptors per group
    bsh = [nc.dram_tensor(f"bsh{k}", [P, V // P, F], BF16, kind="Internal",
                          addr_space="Shared") for k in range(3)]
    bin_d = [nc.dram_tensor(f"bin{k}", [1, 16], BF16, kind="Internal")
             for k in range(3)]
    bout_d = [nc.dram_tensor(f"bout{k}", [2, 16], BF16, kind="Internal")
              for k in range(3)]

    with tile.TileContext(nc) as tc, ExitStack() as ctx:
        const = ctx.enter_context(tc.tile_pool(name="const", bufs=1))
        zres = ctx.enter_context(tc.tile_pool(name="zres", bufs=1))
        xpool = ctx.enter_context(tc.tile_pool(name="x", bufs=2))
        gpool = ctx.enter_context(tc.tile_pool(name="gbuf", bufs=10))
        spool = ctx.enter_context(tc.tile_pool(name="sel", bufs=4))
        opool = ctx.enter_context(tc.tile_pool(name="ostg", bufs=2))
        bpool = ctx.enter_context(tc.tile_pool(name="bounce", bufs=1))
        psz = ctx.enter_context(tc.tile_pool(name="psz", bufs=4, space="PSUM"))
        pss = ctx.enter_context(tc.tile_pool(name="pss", bufs=4, space="PSUM"))

        # constants + metadata resident in SBUF
        iota_sb = const.tile([P, P], BF16, tag="iota")
        nc.sync.dma_start(iota_sb[:], iota_d[:, :])
        ident_sb = const.tile([P, P], BF16, tag="ident")
        nc.sync.dma_start(ident_sb[:], ident_d[:, :])
        nident_sb = const.tile([P, P], BF16, tag="nident")
        nc.sync.dma_start(nident_sb[:], nident_d[:, :])
        ones_sb = const.tile([1, P], BF16, tag="ones")
        nc.sync.dma_start(ones_sb[:], onesb_d[:, :])
        biasw_sb = const.tile([1, KV * FOUT], BF16, tag="biasw")
        nc.sync.dma_start(biasw_sb[:], biasw_d[:, :])
        wz_sb = const.tile([P, 2, KV * FOUT], BF16, tag="wz")
        nc.sync.dma_start(wz_sb[:], wz_d[:, :, :])
        gidx_sb = const.tile([P, nnzp // 16], I16, tag="gidx")
        nc.sync.dma_start(gidx_sb[:], gidx_d[:, :])
        growl_sb = const.tile([P, nchunk], FP32, tag="growl")
        nc.sync.dma_start(growl_sb[:], growl_d[:, :])
        gval1_sb = const.tile([P, nchunk], FP32, tag="gval1")
        nc.sync.dma_start(gval1_sb[:], gval1_d[:, :])
        gval2_sb = const.tile([P, nchunk], FP32, tag="gval2")
        nc.sync.dma_start(gval2_sb[:], gval2_d[:, :])

        # my tile offset into the shared tensors (0 or NT); loaded on both
        # engines that issue symbolic shared writes (SP for the Z-phase b3
        # writes, Activation for the spmm-phase writes)
        off_by_eng = {}
        for eng, nm in ((nc.scalar, "act"), (nc.sync, "sp")):
            off_reg = eng.alloc_register(f"slab_off_{nm}")
            eng.reg_load(off_reg, offt_d[0:1, 0:1])
            off_by_eng[nm] = eng.snap(off_reg, donate=True, min_val=0,
                                      max_val=NT)

        # all z_k resident in SBUF: [P, nt, KV, BG, FOUT] bf16 (96KB/partition)
        z_sb = zres.tile([P, nt, KV, BG, FOUT], BF16, tag="z")

        shared_writes = {0: [], 1: [], 2: []}

        def write_half(kidx, kslot, grp, ntiles=WGRP):
            """Batched write of ntiles tiles of z-slot kslot to shared bsh[kidx]."""
            g0 = grp * WGRP
            eng, off = ((nc.sync, "sp") if kidx == 0 else (nc.scalar, "act"))
            dst = bsh[kidx][:, bass.ds(off_by_eng[off] + g0, ntiles), :]
            src = z_sb[:, g0:g0 + ntiles, kslot, :, :] \
                .rearrange("p t b o -> p t (b o)")
            w = eng.dma_start(dst, src)
            shared_writes[kidx].append(w)

        # ---------- phase Z: z_k = x0 @ w_k (+ bias folded into z0) ----------
        VHH = VH // 2
        for b in range(BG):
          for half in range(2):
            v0 = half * VHH
            xb = xpool.tile([P, 2, VHH], BF16, tag="xb")
            nc.sync.dma_start(
                xb[:], xt_d[b, :, :, v0:v0 + VHH].rearrange("c p v -> p c v"))
            for vt0 in range(half * nt // 2, (half + 1) * nt // 2, 2):
                zps = psz.tile([P, 2, KV * FOUT], FP32, tag="zps")
                for sub in range(2):
                    vt = vt0 + sub
                    for cc in range(2):
                        nc.tensor.matmul(
                            zps[:, sub, :],
                            lhsT=xb[:, cc, vt * P - v0:(vt + 1) * P - v0],
                            rhs=wz_sb[:, cc, :],
                            start=(cc == 0),
                            stop=(cc == 1 and not has_bias))
                    if has_bias:
                        nc.tensor.matmul(zps[:, sub, :], lhsT=ones_sb[:, :],
                                         rhs=biasw_sb[:, :], start=False,
                                         stop=True)
                # PSUM->SBUF cast copies: DVE 1/3, Act 2/3 (Act is cheaper)
                if (vt0 // 2) % 3 == 0:
                    nc.vector.tensor_copy(
                        z_sb[:, vt0:vt0 + 2, :, b, :],
                        zps[:].rearrange("p s (k o) -> p s k o", o=FOUT))
                else:
                    nc.scalar.activation(
                        out=z_sb[:, vt0:vt0 + 2, :, b, :],
                        in_=zps[:].rearrange("p s (k o) -> p s k o", o=FOUT),
                        func=mybir.ActivationFunctionType.Copy)
                if b == BG - 1 and (vt0 + 2) % WGRP == 0:
                    write_half(0, 3, vt0 // WGRP)

        def pair_barrier(k):
            # the AllGather is a pure rendezvous: gate it on ALL my shared
            # writes; completion proves the peer's writes are done too (the
            # payload itself is never read)
            cc = nc.gpsimd.collective_compute(
                "AllGather", mybir.AluOpType.bypass, PAIR_GROUPS,
                ins=[bin_d[k][0:1, :]], outs=[bout_d[k][:, :]])
            for w in shared_writes[k]:
                bass._add_dep_helper(cc.ins, w.ins, sync=True,
                                     reason="barrier after all shared writes")
            return cc

        # ---------- spmm phases ----------
        # Each phase runs in two passes: pass A covers own-half columns
        # (rows this core wrote -> no cross-core barrier; overlaps with the
        # AllGather rendezvous), pass B covers peer-half columns and waits
        # on the barrier. z-slots accumulate partials between the passes.
        chunksA, chunksB = chunks_per_tile
        nA = sum(chunksA)

        def spmm_pass(src_d, vals_sb, base0, chunks_list, nend, dep_inst,
                      seeds, finish):
            state = {"gb": None, "base": base0, "len": 0}

            def ensure_piece(c):
                while state["gb"] is None or c >= state["base"] + state["len"]:
                    base = (base0 if state["gb"] is None
                            else state["base"] + state["len"])
                    plen = min(CHUNKS_PER_PIECE, nend - base)
                    gb = gpool.tile([P, plen, F], BF16, tag="gb")
                    s0 = base * P
                    nidx = plen * P
                    g = nc.gpsimd.dma_gather(
                        out_ap=gb[:],
                        in_ap=src_d[:, :, :].rearrange("p t f -> (p t) f"),
                        idxs_ap=gidx_sb[:, s0 // 16:(s0 + nidx) // 16],
                        num_idxs=nidx,
                        num_idxs_reg=nidx,
                        elem_size=F,
                    )
                    if dep_inst is not None:
                        bass._add_dep_helper(g.ins, dep_inst.ins, sync=True,
                                             reason="pair barrier before gather")
                    state.update(gb=gb, base=base, len=plen)
                return state["gb"], state["base"]

            ci = base0
            for tt in range(nt):
                nck = chunks_list[tt]
                if nck == 0:
                    continue
                ps = pss.tile([P, F], FP32, tag="ps")
                # seed the accumulator with the running z-slot value(s) so
                # the DVE never has to do the adds
                sds = seeds(tt)
                for si, (w, src) in enumerate(sds):
                    nc.tensor.matmul(ps[:], lhsT=w, rhs=src,
                                     start=(si == 0), stop=False)
                for k in range(nck):
                    col = ci + k
                    gb, base = ensure_piece(col)
                    sT = spool.tile([P, P], BF16, tag="sT")
                    nc.vector.tensor_scalar(
                        out=sT[:], in0=iota_sb[:],
                        scalar1=growl_sb[:, col:col + 1],
                        scalar2=vals_sb[:, col:col + 1],
                        op0=mybir.AluOpType.is_equal,
                        op1=mybir.AluOpType.mult,
                    )
                    nc.tensor.matmul(ps[:], lhsT=sT[:], rhs=gb[:, col - base, :],
                                     start=False, stop=(k == nck - 1))
                finish(tt, ps)
                ci += nck

        def spmm_phase(src_d, vals_sb, cc_inst, seedsA, finishA, seedsB,
                       finishB):
            spmm_pass(src_d, vals_sb, 0, chunksA, nA, None, seedsA, finishA)
            spmm_pass(src_d, vals_sb, nA, chunksB, nchunk, cc_inst, seedsB,
                      finishB)

        def zslot(vt, k):
            return z_sb[:, vt, k, :, :].rearrange("p b o -> p (b o)")

        def ps3(ps):
            return ps[:].rearrange("p (b o) -> p b o", o=FOUT)

        def drain(tt, k, ps):
            nc.scalar.activation(out=zslot(tt, k), in_=ps[:],
                                 func=mybir.ActivationFunctionType.Copy)

        # phase 1: b2 = z2 + 2 L b3   (result overwrites z2 slot)
        cc0 = pair_barrier(0)

        def seeds1(tt):
            return [(ident_sb[:], zslot(tt, 2))]

        def finish1A(tt, ps):
            drain(tt, 2, ps)

        def finish1B(tt, ps):
            drain(tt, 2, ps)
            if tt == nt - WGRP // 2 - 1:
                write_half(1, 2, tt // WGRP, WGRP // 2)
            elif tt == nt - 1:
                g0 = nt - WGRP // 2
                w = nc.scalar.dma_start(
                    bsh[1][:, bass.ds(off_by_eng["act"] + g0, WGRP // 2), :],
                    z_sb[:, g0:g0 + WGRP // 2, 2, :, :]
                    .rearrange("p t b o -> p t (b o)"))
                shared_writes[1].append(w)
            elif (tt + 1) % WGRP == 0:
                write_half(1, 2, tt // WGRP)

        spmm_phase(bsh[0], gval2_sb, cc0, seeds1, finish1A, seeds1, finish1B)

        # phase 2: b1 = z1 + 2 L b2 - b3   (result overwrites z1 slot)
        cc1 = pair_barrier(1)

        def seeds2A(tt):
            return [(ident_sb[:], zslot(tt, 1))]

        def seeds2B(tt):
            return [(ident_sb[:], zslot(tt, 1)),
                    (nident_sb[:], zslot(tt, 3))]

        def finish2A(tt, ps):
            drain(tt, 1, ps)

        def finish2B(tt, ps):
            drain(tt, 1, ps)
            if tt == nt - WGRP // 2 - 1:
                write_half(2, 1, tt // WGRP, WGRP // 2)
            elif tt == nt - 1:
                g0 = nt - WGRP // 2
                w = nc.scalar.dma_start(
                    bsh[2][:, bass.ds(off_by_eng["act"] + g0, WGRP // 2), :],
                    z_sb[:, g0:g0 + WGRP // 2, 1, :, :]
                    .rearrange("p t b o -> p t (b o)"))
                shared_writes[2].append(w)
            elif (tt + 1) % WGRP == 0:
                write_half(2, 1, tt // WGRP)

        spmm_phase(bsh[1], gval2_sb, cc1, seeds2A, finish2A, seeds2B, finish2B)

        # phase 3: out = (z0 + L_own b1) + L_peer b1 - b2   (bias already in z0)
        cc2 = pair_barrier(2)

        def seeds3A(tt):
            return [(ident_sb[:], zslot(tt, 0))]

        def seeds3B(tt):
            return [(ident_sb[:], zslot(tt, 0)),
                    (nident_sb[:], zslot(tt, 2))]

        def finish3A(tt, ps):
            drain(tt, 0, ps)

        ostate = {"ot": None}

        def finish3B(tt, ps):
            if tt % WGRP == 0:
                ot_new = opool.tile([P, WGRP, F], BF16, tag="ot")
                ostate["ot"] = ot_new
            ot = ostate["ot"]
            nc.scalar.activation(out=ot[:, tt % WGRP, :], in_=ps[:],
                                 func=mybir.ActivationFunctionType.Copy)
            if (tt + 1) % WGRP == 0:
                g0 = (tt // WGRP) * WGRP
                nc.sync.dma_start(out_d[:, g0:g0 + WGRP, :], ot[:])

        spmm_phase(bsh[2], gval1_sb, cc2, seeds3A, finish3A, seeds3B, finish3B)

    nc.compile()
    return nc


def make_host_inputs(inputs, weight, bias, lap_vals, lap_rows, lap_cols):
    per_parity, chunks = _preprocess_lap(
        np.asarray(lap_rows), np.asarray(lap_cols),
        np.asarray(lap_vals, np.float32))
    w = np.asarray(weight, np.float32)
    # wz[(t,f) split cc, (k,o)]
    wz = np.transpose(w, (2, 0, 1, 3)).reshape(C, KV * FOUT)
    wz = np.ascontiguousarray(
        wz.reshape(2, P, KV * FOUT).transpose(1, 0, 2)).astype(ml_dtypes.bfloat16)
    biasw = np.zeros((1, KV * FOUT), np.float32)
    biasw[0, :FOUT] = np.asarray(bias, np.float32)
    biasw = biasw.astype(ml_dtypes.bfloat16)
    onesb = np.ones((1, P), ml_dtypes.bfloat16)
    ident128 = np.eye(P, dtype=np.float32).astype(ml_dtypes.bfloat16)
    iota128 = np.ascontiguousarray(
        np.broadcast_to(np.arange(P, dtype=np.float32)[None, :],
                        (P, P))).astype(ml_dtypes.bfloat16)
    x = np.asarray(inputs, np.float32)
    in_maps = []
    for r in range(N_CORES):
        pair, h = r // 2, r % 2
        gidx_w, growl_m, gval_m = per_parity[h]
        # xt[b, cc, cl, v] = x[4p+b, h*VH + v, t, f], c=(t,f)=cc*128+cl
        xs = x[BG * pair:BG * (pair + 1), h * VH:(h + 1) * VH]  # [4, VH, T, FIN]
        xt = xs.reshape(BG, VH, C).transpose(0, 2, 1).reshape(BG, 2, P, VH)
        m = {
            "xt": np.ascontiguousarray(xt).astype(ml_dtypes.bfloat16),
            "wz": wz,
            "biasw": biasw,
            "onesb": onesb,
            "iota128": iota128,
            "ident128": ident128,
            "nident128": -ident128,
            "offt": np.array([[h * NT]], np.int32),
            "gidx": gidx_w,
            "growl": growl_m,
            "gval1": gval_m,
            "gval2": np.ascontiguousarray(2.0 * gval_m),
        }
        in_maps.append(m)
    return in_maps, chunks


_CACHE = {}


def _get_program(chunks, has_bias):
    key = (tuple(chunks[0]), tuple(chunks[1]), has_bias)
    if key not in _CACHE:
        _CACHE[key] = build_program((list(chunks[0]), list(chunks[1])), has_bias)
    return _CACHE[key]


def kernel(inputs, weight, bias, lap_vals, lap_rows, lap_cols):
    in_maps, chunks = make_host_inputs(inputs, weight, bias, lap_vals,
                                       lap_rows, lap_cols)
    nc = _get_program(chunks, bool(np.any(np.asarray(bias))))
    res = run_bass_kernel_spmd(nc, in_maps, list(range(N_CORES)))
    out = np.empty((B, V, FOUT), np.float32)
    for r in range(N_CORES):
        pair, h = r // 2, r % 2
        o = np.asarray(res.results[r]["out"], np.float32).reshape(P, NT, BG, FOUT)
        o = o.transpose(2, 1, 0, 3).reshape(BG, VH, FOUT)
        out[BG * pair:BG * (pair + 1), h * VH:(h + 1) * VH, :] = o
    return np.ascontiguousarray(out)


def time_kernel(inputs_dict, iters=3):
    """Wall-clock repeated executions of the cached program (ns per run)."""
    import time

    in_maps, chunks = make_host_inputs(**inputs_dict)
    nc = _get_program(chunks, bool(np.any(np.asarray(inputs_dict["bias"]))))
    times = []
    for _ in range(iters):
        t0 = time.perf_counter()
        run_bass_kernel_spmd(nc, in_maps, list(range(N_CORES)))
        times.append(time.perf_counter() - t0)
    return min(times) * 1e9
